# revision 72
# baseline (speedup 1.0000x reference)
"""Tied-row (MSA) attention on 8 Trainium2 NeuronCores.

Reference computation (B=128, n=512, dim=256, h=8, dh=64,
r=tie_attn_dim=64, b=B//r=2):
    q = x @ Wq ; k,v = split(x @ Wkv)
    dots[b,h,i,j] = sum_{r,d} q[b,r,h,i,d] k[b,r,h,j,d] * scale
    attn = softmax_j(dots)
    out[b,r,h,i,d] = sum_j attn[b,h,i,j] v[b,r,h,j,d]
    y = out @ Wo + bo

Sharding: 8 cores = b(2) x head-pairs(4).  Each core owns one batch
element and 2 of the 8 heads and produces the partial
    y_part = out[:, :, own 2 heads, :] @ Wo[own 128 rows, :]
summed on the host (the head reduction of the output projection
commutes with the sum); bo added once.

Cost-model-driven design (timeline cost = sum over matmuls of OUTPUT
FREE SIZE x 0.4167ns; K and M are free):
  * dots contracts K=128 = (2 MSA rows x 64 d) per accumulation step
    (32 steps instead of 64), halving the dots charge.  q/k are staged
    per-head as [128=(parity,d), 32 rchunk, n] fp16.  Engine copies
    cannot cross partitions, so the projection alternates head-swapped
    weight copies (wq_even / wq_odd with column halves swapped) making
    every split copy partition-identity:
      even r: ps[0:64]=h0 -> q0[0:64], ps[64:128]=h1 -> q1[64:128]
      odd  r: ps[0:64]=h1 -> q1[0:64], ps[64:128]=h0 -> q0[64:128]
    q goes PSUM -> f16 staging (DVE) then two cheap f16 splits (DVE 2x
    + Pool); k copies direct on ACT (Pool cannot read PSUM on walrus).
  * out is computed as [i, (h d)] (lhsT=attnT chunk, rhs=v[j,(h,d)]
    slice) so the charged free axis is d=64: 32 matmuls x 64 free per
    row instead of 8 x 512.  The y projection needs out^T [(h d), i];
    those transposes run on the PE itself (nc.tensor.transpose with a
    DRAM-loaded identity, 53ns per 128x128 fp16 tile) -- no DMA-server
    or cross-queue latency in the B -> y critical chain.
  * y is written fp16 (halves the writeout on the single exclusive
    DMA server, elem=512B full rate); partials summed in f32 on host.
  * one kernel-wide PSUM pool with shared tags (pool-scope closes
    would barrier phase 3 on all softmax reads): dots tag = dotsA +
    phase-3 v/out rotation, q tag = q_ps + t_ps, k tag = k_ps + dotsB
    + y halves.
  * x streams as 4-row half-block DMAs; phase-3 halves 0-5 prefetch
    into phase-1/2 DMA slack, the rest issue 4 ahead (never parking on
    the in-order SP queue); y writeout on the Pool/SWDGE queue keeps
    SP free for x (no head-of-line inversion into PE-critical loads).

Per-core phases (PE charge 232.2us = 218.5 floor + 13.7 PE transpose):
  Phase 1 (r-loop, proj + fused dots wave A for i-tiles 0,1): softmax
    A and its attnT transposes overlap wave B.
  Wave B (i-tiles 2,3) reuses the q/k PSUM banks; softmax B + per-head
    late attnT transposes feed phase 3.
  Phase 3 (stages A=v proj, B=out, T=PE transpose, C=y, lags 9/11/12;
    stage A leads 9 rows so v projections cover the attnT latency at
    entry; rows 0-3 run their i-tile-0/1 halves first).  Last block
    drains in 2-row transposes and 2/1-row y DMAs to shorten the tail.

  Built with bacc.Bacc(): its compile() pass legalizes Tile's sync for
  walrus (which caps sync waits per instruction); callers must
  finalize() the program before running (see _get_program).
"""

import os
import sys

for _p in ("/opt/trn_rl_repo", "/root/.axon_site/_ro/trn_rl_repo"):
    if os.path.isdir(_p) and _p not in sys.path:
        sys.path.insert(0, _p)

import numpy as np

R = 64          # tie dim (MSA rows per batch element)
RB = 8          # rows per DMA block
RC = R // 2     # dots K-chunks (2 rows each)
N = 512         # sequence length
C = 256         # model dim
HP = 128        # head-pair width: 2 heads x 64
E = 256         # output dim
NCORES = 8
CLAG = 8        # phase-3 stage-C lag behind stage B (rows)
XH = 4          # rows per x half-block tile
NXH = R // XH   # 16 x half-blocks

_CACHE = {}


def build_program():
    import concourse.bacc as bacc
    from concourse import mybir
    from concourse.tile import TileContext
    from contextlib import ExitStack

    f32 = mybir.dt.float32
    f16 = mybir.dt.float16

    nc = bacc.Bacc()
    xT = nc.declare_dram_parameter("xT", [R, C, N], f16, isOutput=False)
    wqe = nc.declare_dram_parameter("wqe", [C, HP], f16, isOutput=False)
    wqo = nc.declare_dram_parameter("wqo", [C, HP], f16, isOutput=False)
    wke = nc.declare_dram_parameter("wke", [C, HP], f16, isOutput=False)
    wko = nc.declare_dram_parameter("wko", [C, HP], f16, isOutput=False)
    wv = nc.declare_dram_parameter("wv", [C, HP], f16, isOutput=False)
    wo = nc.declare_dram_parameter("wo", [HP, E], f16, isOutput=False)
    idm = nc.declare_dram_parameter("idm", [128, 128], f16, isOutput=False)
    y = nc.declare_dram_parameter("y", [R, N, E], f16, isOutput=True)

    # xT half-block hb viewed as [p, r_in_half, c_chunk, n]
    xT_hb = xT.rearrange("(hb r) (cc p) n -> hb p r cc n", r=XH, p=128)
    # y block rb viewed as [p, r_in_block, i_tile, e]
    y_blk = y.rearrange("(rb r) (t p) e -> rb p r t e", r=RB, p=128)

    with TileContext(nc) as tc, ExitStack() as ctx:
        singles = ctx.enter_context(tc.tile_pool(name="singles", bufs=1))
        sm = ctx.enter_context(tc.tile_pool(name="sm", bufs=4))
        attntp = ctx.enter_context(tc.tile_pool(name="attntp", bufs=2))
        xpool = ctx.enter_context(tc.tile_pool(name="xpool", bufs=6))
        stp = ctx.enter_context(tc.tile_pool(name="stp", bufs=2))
        # one PSUM pool for the whole kernel: pool-scope closes would force
        # phase-3 bank allocations to wait on ALL phase-1/2 readers; with
        # shared tags phase 3 lands on the banks softmax A freed early
        ps = ctx.enter_context(tc.tile_pool(name="ps", space="PSUM", bufs=2))

        # weights first, one DMA each, split across the two HWDGE queues
        # (SP + ACT) so they land in ~2us; then x block 0 in 2-row slices
        # so the first projection starts ~2us after that.
        wq_sb = [singles.tile([128, 2, HP], f16, name=f"wq{p}") for p in range(2)]
        wk_sb = [singles.tile([128, 2, HP], f16, name=f"wk{p}") for p in range(2)]
        wv_sb = singles.tile([128, 2, HP], f16)
        wo_sb = singles.tile([128, E], f16)
        # x rows 0-3 race in first on both queues, then weights, x half 1,
        # and the phase-3-only wv/wo
        w2 = "(cc p) h -> p cc h"
        x_sb0 = xpool.tile([128, XH, 2, N], f16, tag="x", name="x1_0")
        nc.sync.dma_start(out=x_sb0[:, 0:2], in_=xT_hb[0, :, 0:2])
        nc.scalar.dma_start(out=wq_sb[1], in_=wqo.rearrange(w2, p=128))
        nc.sync.dma_start(out=wq_sb[0], in_=wqe.rearrange(w2, p=128))
        nc.scalar.dma_start(out=wk_sb[1], in_=wko.rearrange(w2, p=128))
        nc.sync.dma_start(out=wk_sb[0], in_=wke.rearrange(w2, p=128))
        nc.scalar.dma_start(out=x_sb0[:, 2:4], in_=xT_hb[0, :, 2:4])
        x_sb1 = xpool.tile([128, XH, 2, N], f16, tag="x", name="x1_1")
        nc.sync.dma_start(out=x_sb1[:, 0:2], in_=xT_hb[1, :, 0:2])
        nc.scalar.dma_start(out=x_sb1[:, 2:4], in_=xT_hb[1, :, 2:4])
        nc.scalar.dma_start(out=wv_sb, in_=wv.rearrange(w2, p=128))
        nc.scalar.dma_start(out=wo_sb, in_=wo[:, :])

        # attnT survives into phase 3: kernel-scoped pool
        # layout [j_in_chunk, it, jc, i_in_tile]
        attnT = [attntp.tile([128, 4, 4, 128], f16, tag="attnT",
                             name=f"attnT_{h}") for h in range(2)]

        def softmax(dots_hit, attn_dst):
            """dots PSUM tile -> normalized f16 attn slice.

            No max-subtraction: dots = q k^T with the 1/(sqrt(dh) sqrt(r))
            scale folded into Wq, so entries are ~N(0,1) and exp cannot
            overflow fp32/fp16."""
            ssum = sm.tile([128, 1], f32, tag="ssum", bufs=8)
            rinv = sm.tile([128, 1], f32, tag="rinv", bufs=8)
            nc.scalar.activation(
                out=attn_dst, in_=dots_hit,
                func=mybir.ActivationFunctionType.Exp,
                accum_out=ssum)
            nc.vector.reciprocal(rinv, ssum)
            nc.vector.tensor_scalar_mul(attn_dst, attn_dst, rinv)

        if True:
            # attn_h [i_in_tile, it, j]; one tile per head so the transpose
            # to attnT_h is a single xbar DMA per head (kernel-scoped pool:
            # a pool close before phase 3 would barrier DVE on the late
            # attn transposes)
            attn = [attntp.tile([128, 4, N], f16, name=f"attn_{h}")
                    for h in range(2)]

            # resident per-head K-packed fp16 q/k, phases 1-2 only
            with tc.tile_pool(name="resid", bufs=1) as resid:
                qh = [resid.tile([128, RC, N], f16, name=f"q{h}")
                      for h in range(2)]
                kh = [resid.tile([128, RC, N], f16, name=f"k{h}")
                      for h in range(2)]

                def dots_chunk(dots_tiles, c, its):
                    for h in range(2):
                        for it in its:
                            nc.tensor.matmul(
                                dots_tiles[h][it % 2],
                                lhsT=qh[h][:, c, it * 128:(it + 1) * 128],
                                rhs=kh[h][:, c, :],
                                start=(c == 0), stop=(c == RC - 1))

                # ---- Phase 1 + dots wave A (i-tiles 0,1) fused;
                # ---- wave B (i-tiles 2,3) reuses the q/k PSUM banks and
                # ---- runs before softmax A is emitted so the ACT exps of
                # ---- wave A overlap wave B's accumulation.
                x3 = {}
                if True:
                    dotsA = [[ps.tile([128, N], f32, tag="dots", bufs=4,
                                       name=f"dotsA_{h}_{it}")
                              for it in range(2)] for h in range(2)]
                    x_sb = x_sb0
                    for r in range(R):
                        hb, ri = divmod(r, XH)
                        if ri == 0 and hb == 1:
                            x_sb = x_sb1
                        elif ri == 0 and hb > 1:
                            x_sb = xpool.tile([128, XH, 2, N], f16, tag="x",
                                              name=f"x1_{hb}")
                            nc.sync.dma_start(out=x_sb, in_=xT_hb[hb])
                        # prefetch phase-3 x halves 0-3 into the slots the
                        # phase-1 stream has finished with
                        if r >= 48 and ri == 0:
                            hb3 = (r - 48) // 4
                            x3[hb3] = xpool.tile([128, XH, 2, N], f16, tag="x",
                                                 name=f"x3_{hb3}")
                            nc.sync.dma_start(out=x3[hb3], in_=xT_hb[hb3])
                        par = r % 2
                        rc = r // 2
                        q_ps = ps.tile([128, N], f32, tag="q")
                        k_ps = ps.tile([128, N], f32, tag="k")
                        for cc in range(2):
                            nc.tensor.matmul(q_ps, lhsT=wq_sb[par][:, cc, :],
                                             rhs=x_sb[:, ri, cc, :],
                                             start=(cc == 0), stop=(cc == 1))
                        for cc in range(2):
                            nc.tensor.matmul(k_ps, lhsT=wk_sb[par][:, cc, :],
                                             rhs=x_sb[:, ri, cc, :],
                                             start=(cc == 0), stop=(cc == 1))
                        # partition-identity split copies (see module doc):
                        # lo half -> head `par`, hi half -> head `1-par`.
                        # q goes PSUM -> f16 staging (DVE 1x) then two cheap
                        # f16 SBUF splits (DVE 2x mode + Pool), keeping every
                        # engine under the phase-1 PE time; k copies direct
                        # on ACT.  (Pool cannot read PSUM on walrus.)
                        lo, hi = (0, 1) if par == 0 else (1, 0)
                        q_st = stp.tile([128, N], f16, tag="qst", name=f"qst_{r}")
                        nc.vector.tensor_copy(q_st, q_ps)
                        nc.vector.tensor_copy(
                            qh[lo][0:64, rc, :], q_st[0:64, :])
                        nc.gpsimd.tensor_copy(
                            qh[hi][64:128, rc, :], q_st[64:128, :])
                        nc.scalar.copy(kh[lo][0:64, rc, :], k_ps[0:64, :])
                        nc.scalar.copy(kh[hi][64:128, rc, :], k_ps[64:128, :])
                        # wave A dots, two rows behind the copies
                        if par == 1 and r >= 3:
                            dots_chunk(dotsA, (r - 3) // 2, (0, 1))
                    dots_chunk(dotsA, RC - 1, (0, 1))

                    # wave B accumulators take over the q/k bank slots
                    dotsB = [[ps.tile([128, N], f32, tag=t, name=f"dotsB_{h}_{it}")
                              for it, t in ((2, "q"), (3, "k"))] for h in range(2)]
                    dotsB = [{2: dotsB[h][0], 3: dotsB[h][1]} for h in range(2)]
                    # wave B runs as two sequential sub-waves so i-tile
                    # 2's softmax + transpose complete ~13us before the wave
                    # ends -- only i-tile 3's short chain stays on the
                    # phase-3 critical path.  Softmax A and its transposes
                    # overlap sub-wave it2 on ACT/DVE/SP.
                    for it in (2, 3):
                        for c in range(RC):
                            for h in range(2):
                                nc.tensor.matmul(
                                    dotsB[h][it],
                                    lhsT=qh[h][:, c, it * 128:(it + 1) * 128],
                                    rhs=kh[h][:, c, :],
                                    start=(c == 0), stop=(c == RC - 1))
                            if it == 2 and c in (4, 20):  # phase-3 x prefetch
                                hb3 = {4: 4, 20: 5}[c]
                                x3[hb3] = xpool.tile([128, XH, 2, N], f16,
                                                     tag="x", name=f"x3_{hb3}")
                                nc.sync.dma_start(out=x3[hb3], in_=xT_hb[hb3])
                        if it == 2:
                            for h in range(2):
                                for ita in range(2):
                                    softmax(dotsA[h][ita], attn[h][:, ita, :])
                            for h in range(2):
                                nc.sync.dma_start_transpose(
                                    out=attnT[h][:, 0:2], in_=attn[h][:, 0:2, :])
                        for h in range(2):
                            softmax(dotsB[h][it], attn[h][:, it, :])
                        for h in range(2):
                            nc.sync.dma_start_transpose(
                                out=attnT[h][:, it:it + 1],
                                in_=attn[h][:, it:it + 1, :])

        # ------- Phase 3: v, out, outT (PE transpose), y ------------------
        # The out^T transposes run on the PE itself (is_transpose matmuls,
        # 53ns per 128x128 fp16 tile): no DMA-server or cross-queue latency
        # in the B -> y critical chain, so stage lags are short.  PSUM: v(2)
        # + out(2) + T fp16(2 half-banks) + 2 y half-tiles = 8 banks.
        with tc.tile_pool(name="vpool", bufs=12) as vpool, \
             tc.tile_pool(name="outp", bufs=5) as outp, \
             tc.tile_pool(name="outtp", bufs=5) as outtp, \
             tc.tile_pool(name="ypool", bufs=2) as ypool:
            ident = singles.tile([128, 128], f16, name="ident")
            nc.scalar.dma_start(out=ident, in_=idm[:, :])
            v_sbs = {}
            out_sbs = {}
            outT_sbs = {}
            y_sbs = {}

            def stage_a(r):
                hb, ri = divmod(r, XH)
                # issue-ahead of 4 halves: the reused buffer slot's readers
                # retired ~9 rows ago, so this DMA never parks on the SP queue
                if ri == 0 and 6 <= hb + 4 < NXH:
                    x3[hb + 4] = xpool.tile([128, XH, 2, N], f16, tag="x",
                                            name=f"x3_{hb + 4}")
                    nc.sync.dma_start(out=x3[hb + 4], in_=xT_hb[hb + 4])
                v_ps = ps.tile([128, 4, 128], f32, tag="dots", bufs=4,
                                name=f"v_ps_{r}")
                for jt in range(4):
                    for cc in range(2):
                        nc.tensor.matmul(
                            v_ps[:, jt, :],
                            lhsT=x3[hb][:, ri, cc, jt * 128:(jt + 1) * 128],
                            rhs=wv_sb[:, cc, :],
                            start=(cc == 0), stop=(cc == 1))
                v_sb = vpool.tile([128, 4, 128], f16, tag="vsb", name=f"v_sb_{r}")
                # entry rows copy on DVE (idle until B(0)) so the A-lead can
                # run ahead while ACT drains softmax B
                if r < 9:
                    nc.vector.tensor_copy(v_sb, v_ps)
                else:
                    nc.scalar.copy(v_sb, v_ps)
                v_sbs[r] = v_sb

            out_pss = {}

            def stage_b(r, its=(0, 1, 2, 3), done=True):
                if r in out_pss:
                    out_ps = out_pss[r]
                else:
                    # entry rows 0-1 park on the q-tag banks (freed by the
                    # early it2 softmax) so the long-lived entry out tiles
                    # don't stall the dots-tag v/out rotation
                    tg, bf = ("q", 2) if r < 2 and its != (0, 1, 2, 3) \
                        else ("dots", 4)
                    out_ps = out_pss[r] = ps.tile(
                        [128, 4, 128], f32, tag=tg, bufs=bf,
                        name=f"out_ps_{r}")
                for it in its:
                    for h in range(2):
                        hs = slice(h * 64, (h + 1) * 64)
                        for jc in range(4):
                            nc.tensor.matmul(
                                out_ps[:, it, hs],
                                lhsT=attnT[h][:, it, jc, :],
                                rhs=v_sbs[r][:, jc, hs],
                                start=(jc == 0), stop=(jc == 3),
                                skip_group_check=True)
                if done:
                    v_sbs.pop(r)
                    out_pss.pop(r)
                    out_sb = outp.tile([128, 4, 128], f16, tag="ob",
                                       name=f"out_sb_{r}")
                    nc.vector.tensor_copy(out_sb, out_ps)
                    out_sbs[r] = out_sb

            def stage_t(r):
                out_sb = out_sbs.pop(r)
                # padded to a full 2KB PSUM bank so the shared "q" tag
                # keeps a single tile size
                t_ps = ps.tile([128, 8, 128], f16, tag="q", name=f"t_ps_{r}")
                for it in range(4):
                    nc.tensor.transpose(t_ps[:, it, :], out_sb[:, it, :], ident)
                outT = outtp.tile([128, 4, 128], f16, tag="ot",
                                  name=f"outT_{r}")
                if r >= R - 4:  # drain: ACT is the serializer, DVE is free
                    nc.vector.tensor_copy(outT, t_ps[:, 0:4, :])
                else:
                    nc.scalar.copy(outT, t_ps[:, 0:4, :])
                outT_sbs[r] = outT

            def stage_c(r):
                rb, ri = divmod(r, RB)
                outT = outT_sbs.pop(r)
                if ri == 0:
                    y_sbs[rb] = ypool.tile([128, RB, 4, E], f16, tag="ysb",
                                           name=f"y_sb_{rb}")
                y_sb = y_sbs[rb]
                # two 1-bank PSUM halves; copies split ACT/Pool (off DVE so
                # out/T copies flow without queueing delay)
                # drain rows borrow the freed dots/q banks so the final
                # C stages double-buffer instead of chaining on 2 banks
                ta, tb = ("dots", "q") if r >= R - 3 else ("k", "k")
                y_psa = ps.tile([128, 2, E], f32, tag=ta,
                                bufs=4 if r >= R - 3 else 2,
                                name=f"y_psa_{r}")
                for it in range(2):
                    nc.tensor.matmul(y_psa[:, it, :], lhsT=outT[:, it, :],
                                     rhs=wo_sb, start=True, stop=True)
                nc.vector.tensor_copy(y_sb[:, ri, 0:2, :], y_psa)
                y_psb = ps.tile([128, 2, E], f32, tag=tb, name=f"y_psb_{r}")
                for it in range(2, 4):
                    nc.tensor.matmul(y_psb[:, it - 2, :], lhsT=outT[:, it, :],
                                     rhs=wo_sb, start=True, stop=True)
                nc.scalar.copy(y_sb[:, ri, 2:4, :], y_psb)
                # half-block writeout on the SWDGE (Pool) queue; the last
                # block drains in 2-row pieces, the final ones on the idle
                # SP/HWDGE queue (~1.2us less SWDGE generation each)
                if rb == RB - 1:
                    pieces = {i: i for i in range(RB)}  # per-row pieces
                else:
                    pieces = {3: 0, 7: 4}
                if ri in pieces:
                    lo = pieces[ri]
                    if rb == RB - 1:
                        # SP/HWDGE for the whole last block: ~1.5us less
                        # SWDGE generation latency per piece at the end
                        nc.sync.dma_start(out=y_blk[rb, :, lo:ri + 1],
                                          in_=y_sb[:, lo:ri + 1])
                    else:
                        nc.gpsimd.dma_start(out=y_blk[rb, :, lo:ri + 1],
                                            in_=y_sb[:, lo:ri + 1])
                    if ri == RB - 1:
                        y_sbs.pop(rb)

            # stage A leads by 9 rows: the v projections (independent of
            # attnT) keep the PE busy through the softmax-B -> attnT
            # transpose latency at phase-3 entry
            for r in range(R + 12):
                if r < R:
                    stage_a(r)
                # entry: i-tile-0/1 halves of rows 0-3 run first (they need
                # only the early attnT transposes), the 2/3 halves catch up
                # two per iteration once the late transposes land
                if r == 9:
                    stage_b(0, its=(0, 1), done=False)
                    stage_b(1, its=(0, 1), done=False)
                elif r == 10:
                    stage_b(2, its=(0, 1), done=False)
                    stage_b(3, its=(0, 1), done=False)
                elif r == 11:
                    # all four it2 halves before any it3 half: the in-order
                    # PE stream then stalls only once, on the it3 attnT
                    for q_ in range(4):
                        stage_b(q_, its=(2,), done=False)
                elif r == 12:
                    for q_ in range(4):
                        stage_b(q_, its=(3,))
                elif 0 <= r - 9 < R:
                    stage_b(r - 9)
                if r == 13:
                    stage_t(0)
                    stage_t(1)
                elif r == 14:
                    stage_t(2)
                    stage_t(3)
                elif 0 <= r - 11 < R and r - 11 >= 4:
                    stage_t(r - 11)
                if r == 14:
                    stage_c(0)
                    stage_c(1)
                elif r == 15:
                    stage_c(2)
                    stage_c(3)
                elif 0 <= r - 12 < R and r - 12 >= 4:
                    stage_c(r - 12)

    return nc


def _get_program():
    if "nc" not in _CACHE:
        nc = build_program()
        nc.finalize()
        _CACHE["nc"] = nc
    return _CACHE["nc"]


def make_in_maps(x, Wq, Wkv, Wo):
    """Host-side sharding: core = bi*4 + hpi."""
    scale = (64.0 ** -0.5) * (64.0 ** -0.5)
    x = np.asarray(x, np.float32)
    Wq = np.asarray(Wq, np.float32) * np.float32(scale)
    Wkv = np.asarray(Wkv, np.float32)
    Wo = np.asarray(Wo, np.float32)
    b = x.shape[0] // R
    xT = np.ascontiguousarray(
        x.reshape(b, R, N, C).transpose(0, 1, 3, 2)).astype(np.float16)

    def swap_heads(w):  # [C, 128] -> column halves swapped
        return np.ascontiguousarray(
            np.concatenate([w[:, 64:], w[:, :64]], axis=1))

    in_maps = []
    for core in range(NCORES):
        bi, hpi = divmod(core, 4)
        cols = slice(hpi * HP, (hpi + 1) * HP)
        wq_c = np.ascontiguousarray(Wq[:, cols]).astype(np.float16)
        wk_c = np.ascontiguousarray(Wkv[:, cols]).astype(np.float16)
        in_maps.append({
            "xT": xT[bi],
            "idm": np.eye(128, dtype=np.float16),
            "wqe": wq_c,
            "wqo": swap_heads(wq_c),
            "wke": wk_c,
            "wko": swap_heads(wk_c),
            "wv": np.ascontiguousarray(
                Wkv[:, 512 + hpi * HP: 512 + (hpi + 1) * HP]).astype(np.float16),
            "wo": np.ascontiguousarray(Wo[cols, :]).astype(np.float16),
        })
    return in_maps


def combine_outputs(ys, bo):
    """ys: list of 8 [R, N, E] partials in core order; returns [B, n, dim]."""
    ys = [np.asarray(t, np.float32) for t in ys]
    y0 = ys[0] + ys[1] + ys[2] + ys[3]
    y1 = ys[4] + ys[5] + ys[6] + ys[7]
    y = np.concatenate([y0, y1], axis=0).reshape(2 * R, N, E)
    return (y + np.asarray(bo, np.float32)).astype(np.float32)


def kernel(x, Wq, Wkv, Wo, bo, tie_attn_dim):
    assert int(tie_attn_dim) == R, f"hardcoded for tie_attn_dim={R}"
    from concourse.bass_utils import run_bass_kernel_spmd

    nc = _get_program()
    in_maps = make_in_maps(x, Wq, Wkv, Wo)
    res = run_bass_kernel_spmd(nc, in_maps, list(range(NCORES)))
    ys = [res.results[c]["y"] for c in range(NCORES)]
    return combine_outputs(ys, bo)


# revision 74
# speedup vs baseline: 1.0006x; 1.0006x over previous
"""Tied-row (MSA) attention on 8 Trainium2 NeuronCores.

Reference computation (B=128, n=512, dim=256, h=8, dh=64,
r=tie_attn_dim=64, b=B//r=2):
    q = x @ Wq ; k,v = split(x @ Wkv)
    dots[b,h,i,j] = sum_{r,d} q[b,r,h,i,d] k[b,r,h,j,d] * scale
    attn = softmax_j(dots)
    out[b,r,h,i,d] = sum_j attn[b,h,i,j] v[b,r,h,j,d]
    y = out @ Wo + bo

Sharding: 8 cores = b(2) x head-pairs(4).  Each core owns one batch
element and 2 of the 8 heads and produces the partial
    y_part = out[:, :, own 2 heads, :] @ Wo[own 128 rows, :]
summed on the host (the head reduction of the output projection
commutes with the sum); bo added once.

Cost-model-driven design (timeline cost = sum over matmuls of OUTPUT
FREE SIZE x 0.4167ns; K and M are free):
  * dots contracts K=128 = (2 MSA rows x 64 d) per accumulation step
    (32 steps instead of 64), halving the dots charge.  q/k are staged
    per-head as [128=(parity,d), 32 rchunk, n] fp16.  Engine copies
    cannot cross partitions, so the projection alternates head-swapped
    weight copies (wq_even / wq_odd with column halves swapped) making
    every split copy partition-identity:
      even r: ps[0:64]=h0 -> q0[0:64], ps[64:128]=h1 -> q1[64:128]
      odd  r: ps[0:64]=h1 -> q1[0:64], ps[64:128]=h0 -> q0[64:128]
    q goes PSUM -> f16 staging (DVE) then two cheap f16 splits (DVE 2x
    + Pool); k copies direct on ACT (Pool cannot read PSUM on walrus).
  * out is computed as [i, (h d)] (lhsT=attnT chunk, rhs=v[j,(h,d)]
    slice) so the charged free axis is d=64: 32 matmuls x 64 free per
    row instead of 8 x 512.  The y projection needs out^T [(h d), i];
    those transposes run on the PE itself (nc.tensor.transpose with a
    DRAM-loaded identity, 53ns per 128x128 fp16 tile) -- no DMA-server
    or cross-queue latency in the B -> y critical chain.
  * y is written fp16 (halves the writeout on the single exclusive
    DMA server, elem=512B full rate); partials summed in f32 on host.
  * one kernel-wide PSUM pool with shared tags (pool-scope closes
    would barrier phase 3 on all softmax reads): dots tag = dotsA +
    phase-3 v/out rotation, q tag = q_ps + t_ps, k tag = k_ps + dotsB
    + y halves.
  * x streams as 4-row half-block DMAs; phase-3 halves 0-5 prefetch
    into phase-1/2 DMA slack, the rest issue 4 ahead (never parking on
    the in-order SP queue); y writeout on the Pool/SWDGE queue keeps
    SP free for x (no head-of-line inversion into PE-critical loads).

Per-core phases (PE charge 232.2us = 218.5 floor + 13.7 PE transpose):
  Phase 1 (r-loop, proj + fused dots wave A for i-tiles 0,1): softmax
    A and its attnT transposes overlap wave B.
  Wave B (i-tiles 2,3) reuses the q/k PSUM banks; softmax B + per-head
    late attnT transposes feed phase 3.
  Phase 3 (stages A=v proj, B=out, T=PE transpose, C=y, lags 9/11/12;
    stage A leads 9 rows so v projections cover the attnT latency at
    entry; rows 0-3 run their i-tile-0/1 halves first).  Last block
    drains in 2-row transposes and 2/1-row y DMAs to shorten the tail.

  Built with bacc.Bacc(): its compile() pass legalizes Tile's sync for
  walrus (which caps sync waits per instruction); callers must
  finalize() the program before running (see _get_program).
"""

import os
import sys

for _p in ("/opt/trn_rl_repo", "/root/.axon_site/_ro/trn_rl_repo"):
    if os.path.isdir(_p) and _p not in sys.path:
        sys.path.insert(0, _p)

import numpy as np

R = 64          # tie dim (MSA rows per batch element)
RB = 8          # rows per DMA block
RC = R // 2     # dots K-chunks (2 rows each)
N = 512         # sequence length
C = 256         # model dim
HP = 128        # head-pair width: 2 heads x 64
E = 256         # output dim
NCORES = 8
CLAG = 8        # phase-3 stage-C lag behind stage B (rows)
XH = 4          # rows per x half-block tile
NXH = R // XH   # 16 x half-blocks

_CACHE = {}


def build_program():
    import concourse.bacc as bacc
    from concourse import mybir
    from concourse.tile import TileContext
    from contextlib import ExitStack

    f32 = mybir.dt.float32
    f16 = mybir.dt.float16

    nc = bacc.Bacc()
    xT = nc.declare_dram_parameter("xT", [R, C, N], f16, isOutput=False)
    wqe = nc.declare_dram_parameter("wqe", [C, HP], f16, isOutput=False)
    wqo = nc.declare_dram_parameter("wqo", [C, HP], f16, isOutput=False)
    wke = nc.declare_dram_parameter("wke", [C, HP], f16, isOutput=False)
    wko = nc.declare_dram_parameter("wko", [C, HP], f16, isOutput=False)
    wv = nc.declare_dram_parameter("wv", [C, HP], f16, isOutput=False)
    wo = nc.declare_dram_parameter("wo", [HP, E], f16, isOutput=False)
    idm = nc.declare_dram_parameter("idm", [128, 128], f16, isOutput=False)
    y = nc.declare_dram_parameter("y", [R, N, E], f16, isOutput=True)

    # xT half-block hb viewed as [p, r_in_half, c_chunk, n]
    xT_hb = xT.rearrange("(hb r) (cc p) n -> hb p r cc n", r=XH, p=128)
    # y block rb viewed as [p, r_in_block, i_tile, e]
    y_blk = y.rearrange("(rb r) (t p) e -> rb p r t e", r=RB, p=128)

    with TileContext(nc) as tc, ExitStack() as ctx:
        singles = ctx.enter_context(tc.tile_pool(name="singles", bufs=1))
        sm = ctx.enter_context(tc.tile_pool(name="sm", bufs=4))
        attntp = ctx.enter_context(tc.tile_pool(name="attntp", bufs=2))
        xpool = ctx.enter_context(tc.tile_pool(name="xpool", bufs=6))
        stp = ctx.enter_context(tc.tile_pool(name="stp", bufs=2))
        # one PSUM pool for the whole kernel: pool-scope closes would force
        # phase-3 bank allocations to wait on ALL phase-1/2 readers; with
        # shared tags phase 3 lands on the banks softmax A freed early
        ps = ctx.enter_context(tc.tile_pool(name="ps", space="PSUM", bufs=2))

        # weights first, one DMA each, split across the two HWDGE queues
        # (SP + ACT) so they land in ~2us; then x block 0 in 2-row slices
        # so the first projection starts ~2us after that.
        wq_sb = [singles.tile([128, 2, HP], f16, name=f"wq{p}") for p in range(2)]
        wk_sb = [singles.tile([128, 2, HP], f16, name=f"wk{p}") for p in range(2)]
        wv_sb = singles.tile([128, 2, HP], f16)
        wo_sb = singles.tile([128, E], f16)
        # x rows 0-3 race in first on both queues, then weights, x half 1,
        # and the phase-3-only wv/wo
        w2 = "(cc p) h -> p cc h"
        x_sb0 = xpool.tile([128, XH, 2, N], f16, tag="x", name="x1_0")
        nc.sync.dma_start(out=x_sb0[:, 0:2], in_=xT_hb[0, :, 0:2])
        nc.scalar.dma_start(out=wq_sb[1], in_=wqo.rearrange(w2, p=128))
        nc.sync.dma_start(out=wq_sb[0], in_=wqe.rearrange(w2, p=128))
        nc.scalar.dma_start(out=wk_sb[1], in_=wko.rearrange(w2, p=128))
        nc.sync.dma_start(out=wk_sb[0], in_=wke.rearrange(w2, p=128))
        nc.scalar.dma_start(out=x_sb0[:, 2:4], in_=xT_hb[0, :, 2:4])
        x_sb1 = xpool.tile([128, XH, 2, N], f16, tag="x", name="x1_1")
        nc.sync.dma_start(out=x_sb1[:, 0:2], in_=xT_hb[1, :, 0:2])
        nc.scalar.dma_start(out=x_sb1[:, 2:4], in_=xT_hb[1, :, 2:4])
        nc.scalar.dma_start(out=wv_sb, in_=wv.rearrange(w2, p=128))
        nc.scalar.dma_start(out=wo_sb, in_=wo[:, :])

        # attnT survives into phase 3: kernel-scoped pool
        # layout [j_in_chunk, it, jc, i_in_tile]
        attnT = [attntp.tile([128, 4, 4, 128], f16, tag="attnT",
                             name=f"attnT_{h}") for h in range(2)]

        def softmax(dots_hit, attn_dst):
            """dots PSUM tile -> normalized f16 attn slice.

            No max-subtraction: dots = q k^T with the 1/(sqrt(dh) sqrt(r))
            scale folded into Wq, so entries are ~N(0,1) and exp cannot
            overflow fp32/fp16."""
            ssum = sm.tile([128, 1], f32, tag="ssum", bufs=8)
            rinv = sm.tile([128, 1], f32, tag="rinv", bufs=8)
            nc.scalar.activation(
                out=attn_dst, in_=dots_hit,
                func=mybir.ActivationFunctionType.Exp,
                accum_out=ssum)
            nc.vector.reciprocal(rinv, ssum)
            nc.vector.tensor_scalar_mul(attn_dst, attn_dst, rinv)

        if True:
            # attn_h [i_in_tile, it, j]; one tile per head so the transpose
            # to attnT_h is a single xbar DMA per head (kernel-scoped pool:
            # a pool close before phase 3 would barrier DVE on the late
            # attn transposes)
            attn = [attntp.tile([128, 4, N], f16, name=f"attn_{h}")
                    for h in range(2)]

            # resident per-head K-packed fp16 q/k, phases 1-2 only
            with tc.tile_pool(name="resid", bufs=1) as resid:
                qh = [resid.tile([128, RC, N], f16, name=f"q{h}")
                      for h in range(2)]
                kh = [resid.tile([128, RC, N], f16, name=f"k{h}")
                      for h in range(2)]

                def dots_chunk(dots_tiles, c, its):
                    for h in range(2):
                        for it in its:
                            nc.tensor.matmul(
                                dots_tiles[h][it % 2],
                                lhsT=qh[h][:, c, it * 128:(it + 1) * 128],
                                rhs=kh[h][:, c, :],
                                start=(c == 0), stop=(c == RC - 1))

                # ---- Phase 1 + dots wave A (i-tiles 0,1) fused;
                # ---- wave B (i-tiles 2,3) reuses the q/k PSUM banks and
                # ---- runs before softmax A is emitted so the ACT exps of
                # ---- wave A overlap wave B's accumulation.
                x3 = {}
                if True:
                    dotsA = [[ps.tile([128, N], f32, tag="dots", bufs=4,
                                       name=f"dotsA_{h}_{it}")
                              for it in range(2)] for h in range(2)]
                    x_sb = x_sb0
                    for r in range(R):
                        hb, ri = divmod(r, XH)
                        if ri == 0 and hb == 1:
                            x_sb = x_sb1
                        elif ri == 0 and hb > 1:
                            x_sb = xpool.tile([128, XH, 2, N], f16, tag="x",
                                              name=f"x1_{hb}")
                            nc.sync.dma_start(out=x_sb, in_=xT_hb[hb])
                        # prefetch phase-3 x halves 0-3 into the slots the
                        # phase-1 stream has finished with
                        if r >= 48 and ri == 0:
                            hb3 = (r - 48) // 4
                            x3[hb3] = xpool.tile([128, XH, 2, N], f16, tag="x",
                                                 name=f"x3_{hb3}")
                            nc.sync.dma_start(out=x3[hb3], in_=xT_hb[hb3])
                        par = r % 2
                        rc = r // 2
                        q_ps = ps.tile([128, N], f32, tag="q")
                        k_ps = ps.tile([128, N], f32, tag="k")
                        for cc in range(2):
                            nc.tensor.matmul(q_ps, lhsT=wq_sb[par][:, cc, :],
                                             rhs=x_sb[:, ri, cc, :],
                                             start=(cc == 0), stop=(cc == 1))
                        for cc in range(2):
                            nc.tensor.matmul(k_ps, lhsT=wk_sb[par][:, cc, :],
                                             rhs=x_sb[:, ri, cc, :],
                                             start=(cc == 0), stop=(cc == 1))
                        # partition-identity split copies (see module doc):
                        # lo half -> head `par`, hi half -> head `1-par`.
                        # q goes PSUM -> f16 staging (DVE 1x) then two cheap
                        # f16 SBUF splits (DVE 2x mode + Pool), keeping every
                        # engine under the phase-1 PE time; k copies direct
                        # on ACT.  (Pool cannot read PSUM on walrus.)
                        lo, hi = (0, 1) if par == 0 else (1, 0)
                        q_st = stp.tile([128, N], f16, tag="qst", name=f"qst_{r}")
                        nc.vector.tensor_copy(q_st, q_ps)
                        nc.vector.tensor_copy(
                            qh[lo][0:64, rc, :], q_st[0:64, :])
                        nc.gpsimd.tensor_copy(
                            qh[hi][64:128, rc, :], q_st[64:128, :])
                        nc.scalar.copy(kh[lo][0:64, rc, :], k_ps[0:64, :])
                        nc.scalar.copy(kh[hi][64:128, rc, :], k_ps[64:128, :])
                        # wave A dots, two rows behind the copies
                        if par == 1 and r >= 3:
                            dots_chunk(dotsA, (r - 3) // 2, (0, 1))
                    dots_chunk(dotsA, RC - 1, (0, 1))

                    # wave B accumulators take over the q/k bank slots
                    dotsB = [[ps.tile([128, N], f32, tag=t, name=f"dotsB_{h}_{it}")
                              for it, t in ((2, "q"), (3, "k"))] for h in range(2)]
                    dotsB = [{2: dotsB[h][0], 3: dotsB[h][1]} for h in range(2)]
                    # wave B runs as two sequential sub-waves so i-tile
                    # 2's softmax + transpose complete ~13us before the wave
                    # ends -- only i-tile 3's short chain stays on the
                    # phase-3 critical path.  Softmax A and its transposes
                    # overlap sub-wave it2 on ACT/DVE/SP.
                    for it in (2, 3):
                        for c in range(RC):
                            for h in range(2):
                                nc.tensor.matmul(
                                    dotsB[h][it],
                                    lhsT=qh[h][:, c, it * 128:(it + 1) * 128],
                                    rhs=kh[h][:, c, :],
                                    start=(c == 0), stop=(c == RC - 1))
                            if it == 2 and c in (4, 20):  # phase-3 x prefetch
                                hb3 = {4: 4, 20: 5}[c]
                                x3[hb3] = xpool.tile([128, XH, 2, N], f16,
                                                     tag="x", name=f"x3_{hb3}")
                                nc.sync.dma_start(out=x3[hb3], in_=xT_hb[hb3])
                        if it == 2:
                            for h in range(2):
                                for ita in range(2):
                                    softmax(dotsA[h][ita], attn[h][:, ita, :])
                            for h in range(2):
                                nc.sync.dma_start_transpose(
                                    out=attnT[h][:, 0:2], in_=attn[h][:, 0:2, :])
                        for h in range(2):
                            softmax(dotsB[h][it], attn[h][:, it, :])
                        for h in range(2):
                            nc.sync.dma_start_transpose(
                                out=attnT[h][:, it:it + 1],
                                in_=attn[h][:, it:it + 1, :])

        # ------- Phase 3: v, out, outT (PE transpose), y ------------------
        # The out^T transposes run on the PE itself (is_transpose matmuls,
        # 53ns per 128x128 fp16 tile): no DMA-server or cross-queue latency
        # in the B -> y critical chain, so stage lags are short.  PSUM: v(2)
        # + out(2) + T fp16(2 half-banks) + 2 y half-tiles = 8 banks.
        with tc.tile_pool(name="vpool", bufs=12) as vpool, \
             tc.tile_pool(name="outp", bufs=5) as outp, \
             tc.tile_pool(name="outtp", bufs=5) as outtp, \
             tc.tile_pool(name="ypool", bufs=2) as ypool:
            ident = singles.tile([128, 128], f16, name="ident")
            nc.scalar.dma_start(out=ident, in_=idm[:, :])
            v_sbs = {}
            out_sbs = {}
            outT_sbs = {}
            y_sbs = {}

            def stage_a(r):
                hb, ri = divmod(r, XH)
                # issue-ahead of 4 halves: the reused buffer slot's readers
                # retired ~9 rows ago, so this DMA never parks on the SP queue
                if ri == 0 and 6 <= hb + 4 < NXH:
                    x3[hb + 4] = xpool.tile([128, XH, 2, N], f16, tag="x",
                                            name=f"x3_{hb + 4}")
                    nc.sync.dma_start(out=x3[hb + 4], in_=xT_hb[hb + 4])
                v_ps = ps.tile([128, 4, 128], f32, tag="dots", bufs=4,
                                name=f"v_ps_{r}")
                for jt in range(4):
                    for cc in range(2):
                        nc.tensor.matmul(
                            v_ps[:, jt, :],
                            lhsT=x3[hb][:, ri, cc, jt * 128:(jt + 1) * 128],
                            rhs=wv_sb[:, cc, :],
                            start=(cc == 0), stop=(cc == 1))
                v_sb = vpool.tile([128, 4, 128], f16, tag="vsb", name=f"v_sb_{r}")
                # entry rows copy on DVE (idle until B(0)) so the A-lead can
                # run ahead while ACT drains softmax B
                if r < 9:
                    nc.vector.tensor_copy(v_sb, v_ps)
                else:
                    nc.scalar.copy(v_sb, v_ps)
                v_sbs[r] = v_sb

            out_pss = {}

            def stage_b(r, its=(0, 1, 2, 3), done=True):
                if r in out_pss:
                    out_ps = out_pss[r]
                else:
                    # entry rows 0-1 park on the q-tag banks (freed by the
                    # early it2 softmax) so the long-lived entry out tiles
                    # don't stall the dots-tag v/out rotation
                    tg, bf = ("q", 2) if r < 2 and its != (0, 1, 2, 3) \
                        else ("dots", 4)
                    out_ps = out_pss[r] = ps.tile(
                        [128, 4, 128], f32, tag=tg, bufs=bf,
                        name=f"out_ps_{r}")
                for it in its:
                    for h in range(2):
                        hs = slice(h * 64, (h + 1) * 64)
                        for jc in range(4):
                            nc.tensor.matmul(
                                out_ps[:, it, hs],
                                lhsT=attnT[h][:, it, jc, :],
                                rhs=v_sbs[r][:, jc, hs],
                                start=(jc == 0), stop=(jc == 3),
                                skip_group_check=True)
                if done:
                    v_sbs.pop(r)
                    out_pss.pop(r)
                    out_sb = outp.tile([128, 4, 128], f16, tag="ob",
                                       name=f"out_sb_{r}")
                    nc.vector.tensor_copy(out_sb, out_ps)
                    out_sbs[r] = out_sb

            def stage_t(r):
                out_sb = out_sbs.pop(r)
                # padded to a full 2KB PSUM bank so the shared "q" tag
                # keeps a single tile size
                t_ps = ps.tile([128, 8, 128], f16, tag="q", name=f"t_ps_{r}")
                for it in range(4):
                    nc.tensor.transpose(t_ps[:, it, :], out_sb[:, it, :], ident)
                outT = outtp.tile([128, 4, 128], f16, tag="ot",
                                  name=f"outT_{r}")
                if r >= R - 4:  # drain: ACT is the serializer, DVE is free
                    nc.vector.tensor_copy(outT, t_ps[:, 0:4, :])
                else:
                    nc.scalar.copy(outT, t_ps[:, 0:4, :])
                outT_sbs[r] = outT

            def stage_c(r):
                rb, ri = divmod(r, RB)
                outT = outT_sbs.pop(r)
                if ri == 0:
                    y_sbs[rb] = ypool.tile([128, RB, 4, E], f16, tag="ysb",
                                           name=f"y_sb_{rb}")
                y_sb = y_sbs[rb]
                # two 1-bank PSUM halves; copies split ACT/Pool (off DVE so
                # out/T copies flow without queueing delay)
                # drain rows borrow the freed dots/q banks so the final
                # C stages double-buffer instead of chaining on 2 banks
                ta, tb = ("dots", "q") if r >= R - 3 else ("k", "k")
                y_psa = ps.tile([128, 2, E], f32, tag=ta,
                                bufs=4 if r >= R - 3 else 2,
                                name=f"y_psa_{r}")
                for it in range(2):
                    nc.tensor.matmul(y_psa[:, it, :], lhsT=outT[:, it, :],
                                     rhs=wo_sb, start=True, stop=True)
                nc.vector.tensor_copy(y_sb[:, ri, 0:2, :], y_psa)
                y_psb = ps.tile([128, 2, E], f32, tag=tb, name=f"y_psb_{r}")
                for it in range(2, 4):
                    nc.tensor.matmul(y_psb[:, it - 2, :], lhsT=outT[:, it, :],
                                     rhs=wo_sb, start=True, stop=True)
                nc.scalar.copy(y_sb[:, ri, 2:4, :], y_psb)
                # half-block writeout on the SWDGE (Pool) queue; the last
                # block drains in 2-row pieces, the final ones on the idle
                # SP/HWDGE queue (~1.2us less SWDGE generation each)
                if rb == RB - 1:
                    pieces = {i: i for i in range(RB)}  # per-row pieces
                else:
                    pieces = {3: 0, 7: 4}
                if ri in pieces:
                    lo = pieces[ri]
                    if rb == RB - 1:
                        # SP/HWDGE for the whole last block: ~1.5us less
                        # SWDGE generation latency per piece at the end
                        nc.sync.dma_start(out=y_blk[rb, :, lo:ri + 1],
                                          in_=y_sb[:, lo:ri + 1])
                    else:
                        nc.gpsimd.dma_start(out=y_blk[rb, :, lo:ri + 1],
                                            in_=y_sb[:, lo:ri + 1])
                    if ri == RB - 1:
                        y_sbs.pop(rb)

            # stage A leads by 9 rows: the v projections (independent of
            # attnT) keep the PE busy through the softmax-B -> attnT
            # transpose latency at phase-3 entry
            for r in range(R + 12):
                if r < R:
                    stage_a(r)
                # entry: i-tile-0/1 halves of rows 0-3 run first (they need
                # only the early attnT transposes), the 2/3 halves catch up
                # two per iteration once the late transposes land
                if r == 9:
                    stage_b(0, its=(0, 1), done=False)
                    stage_b(1, its=(0, 1), done=False)
                elif r == 10:
                    stage_b(2, its=(0, 1), done=False)
                    stage_b(3, its=(0, 1), done=False)
                elif r == 11:
                    # all four it2 halves before any it3 half: the in-order
                    # PE stream then stalls only once, on the it3 attnT
                    for q_ in range(4):
                        stage_b(q_, its=(2,), done=False)
                elif r == 12:
                    for q_ in range(4):
                        stage_b(q_, its=(3,))
                elif 0 <= r - 9 < R:
                    stage_b(r - 9)
                if r == 13:
                    stage_t(0)
                    stage_t(1)
                elif r == 14:
                    stage_t(2)
                    stage_t(3)
                elif 0 <= r - 11 < R - 4 and r - 11 >= 4:
                    stage_t(r - 11)
                if R - 4 <= r - 10 < R:
                    stage_t(r - 10)  # tail rows: 1-iter shorter lag
                if r == 14:
                    stage_c(0)
                    stage_c(1)
                elif r == 15:
                    stage_c(2)
                    stage_c(3)
                elif 0 <= r - 12 < R - 4 and r - 12 >= 4:
                    stage_c(r - 12)
                if R - 4 <= r - 11 < R:
                    stage_c(r - 11)

    return nc


def _get_program():
    if "nc" not in _CACHE:
        nc = build_program()
        nc.finalize()
        _CACHE["nc"] = nc
    return _CACHE["nc"]


def make_in_maps(x, Wq, Wkv, Wo):
    """Host-side sharding: core = bi*4 + hpi."""
    scale = (64.0 ** -0.5) * (64.0 ** -0.5)
    x = np.asarray(x, np.float32)
    Wq = np.asarray(Wq, np.float32) * np.float32(scale)
    Wkv = np.asarray(Wkv, np.float32)
    Wo = np.asarray(Wo, np.float32)
    b = x.shape[0] // R
    xT = np.ascontiguousarray(
        x.reshape(b, R, N, C).transpose(0, 1, 3, 2)).astype(np.float16)

    def swap_heads(w):  # [C, 128] -> column halves swapped
        return np.ascontiguousarray(
            np.concatenate([w[:, 64:], w[:, :64]], axis=1))

    in_maps = []
    for core in range(NCORES):
        bi, hpi = divmod(core, 4)
        cols = slice(hpi * HP, (hpi + 1) * HP)
        wq_c = np.ascontiguousarray(Wq[:, cols]).astype(np.float16)
        wk_c = np.ascontiguousarray(Wkv[:, cols]).astype(np.float16)
        in_maps.append({
            "xT": xT[bi],
            "idm": np.eye(128, dtype=np.float16),
            "wqe": wq_c,
            "wqo": swap_heads(wq_c),
            "wke": wk_c,
            "wko": swap_heads(wk_c),
            "wv": np.ascontiguousarray(
                Wkv[:, 512 + hpi * HP: 512 + (hpi + 1) * HP]).astype(np.float16),
            "wo": np.ascontiguousarray(Wo[cols, :]).astype(np.float16),
        })
    return in_maps


def combine_outputs(ys, bo):
    """ys: list of 8 [R, N, E] partials in core order; returns [B, n, dim]."""
    ys = [np.asarray(t, np.float32) for t in ys]
    y0 = ys[0] + ys[1] + ys[2] + ys[3]
    y1 = ys[4] + ys[5] + ys[6] + ys[7]
    y = np.concatenate([y0, y1], axis=0).reshape(2 * R, N, E)
    return (y + np.asarray(bo, np.float32)).astype(np.float32)


def kernel(x, Wq, Wkv, Wo, bo, tie_attn_dim):
    assert int(tie_attn_dim) == R, f"hardcoded for tie_attn_dim={R}"
    from concourse.bass_utils import run_bass_kernel_spmd

    nc = _get_program()
    in_maps = make_in_maps(x, Wq, Wkv, Wo)
    res = run_bass_kernel_spmd(nc, in_maps, list(range(NCORES)))
    ys = [res.results[c]["y"] for c in range(NCORES)]
    return combine_outputs(ys, bo)


# revision 78
# speedup vs baseline: 1.0007x; 1.0001x over previous
"""Tied-row (MSA) attention on 8 Trainium2 NeuronCores.

Reference computation (B=128, n=512, dim=256, h=8, dh=64,
r=tie_attn_dim=64, b=B//r=2):
    q = x @ Wq ; k,v = split(x @ Wkv)
    dots[b,h,i,j] = sum_{r,d} q[b,r,h,i,d] k[b,r,h,j,d] * scale
    attn = softmax_j(dots)
    out[b,r,h,i,d] = sum_j attn[b,h,i,j] v[b,r,h,j,d]
    y = out @ Wo + bo

Sharding: 8 cores = b(2) x head-pairs(4).  Each core owns one batch
element and 2 of the 8 heads and produces the partial
    y_part = out[:, :, own 2 heads, :] @ Wo[own 128 rows, :]
summed on the host (the head reduction of the output projection
commutes with the sum); bo added once.

Cost-model-driven design (timeline cost = sum over matmuls of OUTPUT
FREE SIZE x 0.4167ns; K and M are free):
  * dots contracts K=128 = (2 MSA rows x 64 d) per accumulation step
    (32 steps instead of 64), halving the dots charge.  q/k are staged
    per-head as [128=(parity,d), 32 rchunk, n] fp16.  Engine copies
    cannot cross partitions, so the projection alternates head-swapped
    weight copies (wq_even / wq_odd with column halves swapped) making
    every split copy partition-identity:
      even r: ps[0:64]=h0 -> q0[0:64], ps[64:128]=h1 -> q1[64:128]
      odd  r: ps[0:64]=h1 -> q1[0:64], ps[64:128]=h0 -> q0[64:128]
    q goes PSUM -> f16 staging (DVE) then two cheap f16 splits (DVE 2x
    + Pool); k copies direct on ACT (Pool cannot read PSUM on walrus).
  * out is computed as [i, (h d)] (lhsT=attnT chunk, rhs=v[j,(h,d)]
    slice) so the charged free axis is d=64: 32 matmuls x 64 free per
    row instead of 8 x 512.  The y projection needs out^T [(h d), i];
    those transposes run on the PE itself (nc.tensor.transpose with a
    DRAM-loaded identity, 53ns per 128x128 fp16 tile) -- no DMA-server
    or cross-queue latency in the B -> y critical chain.
  * y is written fp16 (halves the writeout on the single exclusive
    DMA server, elem=512B full rate); partials summed in f32 on host.
  * one kernel-wide PSUM pool with shared tags (pool-scope closes
    would barrier phase 3 on all softmax reads): dots tag = dotsA +
    phase-3 v/out rotation, q tag = q_ps + t_ps, k tag = k_ps + dotsB
    + y halves.
  * x streams as 4-row half-block DMAs; phase-3 halves 0-5 prefetch
    into phase-1/2 DMA slack, the rest issue 4 ahead (never parking on
    the in-order SP queue); y writeout on the Pool/SWDGE queue keeps
    SP free for x (no head-of-line inversion into PE-critical loads).

Per-core phases (PE charge 232.2us = 218.5 floor + 13.7 PE transpose):
  Phase 1 (r-loop, proj + fused dots wave A for i-tiles 0,1): softmax
    A and its attnT transposes overlap wave B.
  Wave B (i-tiles 2,3) reuses the q/k PSUM banks; softmax B + per-head
    late attnT transposes feed phase 3.
  Phase 3 (stages A=v proj, B=out, T=PE transpose, C=y, lags 9/11/12;
    stage A leads 9 rows so v projections cover the attnT latency at
    entry; rows 0-3 run their i-tile-0/1 halves first).  Last block
    drains in 2-row transposes and 2/1-row y DMAs to shorten the tail.

  Built with bacc.Bacc(): its compile() pass legalizes Tile's sync for
  walrus (which caps sync waits per instruction); callers must
  finalize() the program before running (see _get_program).
"""

import os
import sys

for _p in ("/opt/trn_rl_repo", "/root/.axon_site/_ro/trn_rl_repo"):
    if os.path.isdir(_p) and _p not in sys.path:
        sys.path.insert(0, _p)

import numpy as np

R = 64          # tie dim (MSA rows per batch element)
RB = 8          # rows per DMA block
RC = R // 2     # dots K-chunks (2 rows each)
N = 512         # sequence length
C = 256         # model dim
HP = 128        # head-pair width: 2 heads x 64
E = 256         # output dim
NCORES = 8
CLAG = 8        # phase-3 stage-C lag behind stage B (rows)
XH = 4          # rows per x half-block tile
NXH = R // XH   # 16 x half-blocks

_CACHE = {}


def build_program():
    import concourse.bacc as bacc
    from concourse import mybir
    from concourse.tile import TileContext
    from contextlib import ExitStack

    f32 = mybir.dt.float32
    f16 = mybir.dt.float16

    nc = bacc.Bacc()
    xT = nc.declare_dram_parameter("xT", [R, C, N], f16, isOutput=False)
    wqe = nc.declare_dram_parameter("wqe", [C, HP], f16, isOutput=False)
    wqo = nc.declare_dram_parameter("wqo", [C, HP], f16, isOutput=False)
    wke = nc.declare_dram_parameter("wke", [C, HP], f16, isOutput=False)
    wko = nc.declare_dram_parameter("wko", [C, HP], f16, isOutput=False)
    wv = nc.declare_dram_parameter("wv", [C, HP], f16, isOutput=False)
    wo = nc.declare_dram_parameter("wo", [HP, E], f16, isOutput=False)
    idm = nc.declare_dram_parameter("idm", [128, 128], f16, isOutput=False)
    y = nc.declare_dram_parameter("y", [R, N, E], f16, isOutput=True)

    # xT half-block hb viewed as [p, r_in_half, c_chunk, n]
    xT_hb = xT.rearrange("(hb r) (cc p) n -> hb p r cc n", r=XH, p=128)
    # y block rb viewed as [p, r_in_block, i_tile, e]
    y_blk = y.rearrange("(rb r) (t p) e -> rb p r t e", r=RB, p=128)

    with TileContext(nc) as tc, ExitStack() as ctx:
        singles = ctx.enter_context(tc.tile_pool(name="singles", bufs=1))
        sm = ctx.enter_context(tc.tile_pool(name="sm", bufs=4))
        attntp = ctx.enter_context(tc.tile_pool(name="attntp", bufs=2))
        xpool = ctx.enter_context(tc.tile_pool(name="xpool", bufs=6))
        stp = ctx.enter_context(tc.tile_pool(name="stp", bufs=2))
        # one PSUM pool for the whole kernel: pool-scope closes would force
        # phase-3 bank allocations to wait on ALL phase-1/2 readers; with
        # shared tags phase 3 lands on the banks softmax A freed early
        ps = ctx.enter_context(tc.tile_pool(name="ps", space="PSUM", bufs=2))

        # weights first, one DMA each, split across the two HWDGE queues
        # (SP + ACT) so they land in ~2us; then x block 0 in 2-row slices
        # so the first projection starts ~2us after that.
        wq_sb = [singles.tile([128, 2, HP], f16, name=f"wq{p}") for p in range(2)]
        wk_sb = [singles.tile([128, 2, HP], f16, name=f"wk{p}") for p in range(2)]
        wv_sb = singles.tile([128, 2, HP], f16)
        wo_sb = singles.tile([128, E], f16)
        # x rows 0-3 race in first on both queues, then weights, x half 1,
        # and the phase-3-only wv/wo
        w2 = "(cc p) h -> p cc h"
        x_sb0 = xpool.tile([128, XH, 2, N], f16, tag="x", name="x1_0")
        nc.sync.dma_start(out=x_sb0[:, 0:2], in_=xT_hb[0, :, 0:2])
        nc.scalar.dma_start(out=wq_sb[1], in_=wqo.rearrange(w2, p=128))
        nc.sync.dma_start(out=wq_sb[0], in_=wqe.rearrange(w2, p=128))
        nc.scalar.dma_start(out=wk_sb[1], in_=wko.rearrange(w2, p=128))
        nc.sync.dma_start(out=wk_sb[0], in_=wke.rearrange(w2, p=128))
        nc.scalar.dma_start(out=x_sb0[:, 2:4], in_=xT_hb[0, :, 2:4])
        x_sb1 = xpool.tile([128, XH, 2, N], f16, tag="x", name="x1_1")
        nc.sync.dma_start(out=x_sb1[:, 0:2], in_=xT_hb[1, :, 0:2])
        nc.scalar.dma_start(out=x_sb1[:, 2:4], in_=xT_hb[1, :, 2:4])
        nc.scalar.dma_start(out=wv_sb, in_=wv.rearrange(w2, p=128))
        nc.scalar.dma_start(out=wo_sb, in_=wo[:, :])

        # attnT survives into phase 3: kernel-scoped pool
        # layout [j_in_chunk, it, jc, i_in_tile]
        attnT = [attntp.tile([128, 4, 4, 128], f16, tag="attnT",
                             name=f"attnT_{h}") for h in range(2)]

        def softmax(dots_hit, attn_dst):
            """dots PSUM tile -> normalized f16 attn slice.

            No max-subtraction: dots = q k^T with the 1/(sqrt(dh) sqrt(r))
            scale folded into Wq, so entries are ~N(0,1) and exp cannot
            overflow fp32/fp16."""
            ssum = sm.tile([128, 1], f32, tag="ssum", bufs=8)
            rinv = sm.tile([128, 1], f32, tag="rinv", bufs=8)
            nc.scalar.activation(
                out=attn_dst, in_=dots_hit,
                func=mybir.ActivationFunctionType.Exp,
                accum_out=ssum)
            nc.vector.reciprocal(rinv, ssum)
            nc.vector.tensor_scalar_mul(attn_dst, attn_dst, rinv)

        if True:
            # attn_h [i_in_tile, it, j]; one tile per head so the transpose
            # to attnT_h is a single xbar DMA per head (kernel-scoped pool:
            # a pool close before phase 3 would barrier DVE on the late
            # attn transposes)
            attn = [attntp.tile([128, 4, N], f16, name=f"attn_{h}")
                    for h in range(2)]

            # resident per-head K-packed fp16 q/k, phases 1-2 only
            with tc.tile_pool(name="resid", bufs=1) as resid:
                qh = [resid.tile([128, RC, N], f16, name=f"q{h}")
                      for h in range(2)]
                kh = [resid.tile([128, RC, N], f16, name=f"k{h}")
                      for h in range(2)]

                def dots_chunk(dots_tiles, c, its):
                    for h in range(2):
                        for it in its:
                            nc.tensor.matmul(
                                dots_tiles[h][it % 2],
                                lhsT=qh[h][:, c, it * 128:(it + 1) * 128],
                                rhs=kh[h][:, c, :],
                                start=(c == 0), stop=(c == RC - 1))

                # ---- Phase 1 + dots wave A (i-tiles 0,1) fused;
                # ---- wave B (i-tiles 2,3) reuses the q/k PSUM banks and
                # ---- runs before softmax A is emitted so the ACT exps of
                # ---- wave A overlap wave B's accumulation.
                x3 = {}
                if True:
                    dotsA = [[ps.tile([128, N], f32, tag="dots", bufs=4,
                                       name=f"dotsA_{h}_{it}")
                              for it in range(2)] for h in range(2)]
                    x_sb = x_sb0
                    for r in range(R):
                        hb, ri = divmod(r, XH)
                        if ri == 0 and hb == 1:
                            x_sb = x_sb1
                        elif ri == 0 and hb > 1:
                            x_sb = xpool.tile([128, XH, 2, N], f16, tag="x",
                                              name=f"x1_{hb}")
                            nc.sync.dma_start(out=x_sb, in_=xT_hb[hb])
                        # prefetch phase-3 x halves 0-3 into the slots the
                        # phase-1 stream has finished with
                        if r >= 48 and ri == 0:
                            hb3 = (r - 48) // 4
                            x3[hb3] = xpool.tile([128, XH, 2, N], f16, tag="x",
                                                 name=f"x3_{hb3}")
                            nc.sync.dma_start(out=x3[hb3], in_=xT_hb[hb3])
                        par = r % 2
                        rc = r // 2
                        q_ps = ps.tile([128, N], f32, tag="q")
                        k_ps = ps.tile([128, N], f32, tag="k")
                        for cc in range(2):
                            nc.tensor.matmul(q_ps, lhsT=wq_sb[par][:, cc, :],
                                             rhs=x_sb[:, ri, cc, :],
                                             start=(cc == 0), stop=(cc == 1))
                        for cc in range(2):
                            nc.tensor.matmul(k_ps, lhsT=wk_sb[par][:, cc, :],
                                             rhs=x_sb[:, ri, cc, :],
                                             start=(cc == 0), stop=(cc == 1))
                        # partition-identity split copies (see module doc):
                        # lo half -> head `par`, hi half -> head `1-par`.
                        # q goes PSUM -> f16 staging (DVE 1x) then two cheap
                        # f16 SBUF splits (DVE 2x mode + Pool), keeping every
                        # engine under the phase-1 PE time; k copies direct
                        # on ACT.  (Pool cannot read PSUM on walrus.)
                        lo, hi = (0, 1) if par == 0 else (1, 0)
                        q_st = stp.tile([128, N], f16, tag="qst", name=f"qst_{r}")
                        nc.vector.tensor_copy(q_st, q_ps)
                        nc.vector.tensor_copy(
                            qh[lo][0:64, rc, :], q_st[0:64, :])
                        nc.gpsimd.tensor_copy(
                            qh[hi][64:128, rc, :], q_st[64:128, :])
                        nc.scalar.copy(kh[lo][0:64, rc, :], k_ps[0:64, :])
                        nc.scalar.copy(kh[hi][64:128, rc, :], k_ps[64:128, :])
                        # wave A dots, two rows behind the copies
                        if par == 1 and r >= 3:
                            dots_chunk(dotsA, (r - 3) // 2, (0, 1))
                    dots_chunk(dotsA, RC - 1, (0, 1))

                    # wave B accumulators take over the q/k bank slots
                    dotsB = [[ps.tile([128, N], f32, tag=t, name=f"dotsB_{h}_{it}")
                              for it, t in ((2, "q"), (3, "k"))] for h in range(2)]
                    dotsB = [{2: dotsB[h][0], 3: dotsB[h][1]} for h in range(2)]
                    # wave B runs as two sequential sub-waves so i-tile
                    # 2's softmax + transpose complete ~13us before the wave
                    # ends -- only i-tile 3's short chain stays on the
                    # phase-3 critical path.  Softmax A and its transposes
                    # overlap sub-wave it2 on ACT/DVE/SP.
                    for it in (2, 3):
                        for c in range(RC):
                            for h in range(2):
                                nc.tensor.matmul(
                                    dotsB[h][it],
                                    lhsT=qh[h][:, c, it * 128:(it + 1) * 128],
                                    rhs=kh[h][:, c, :],
                                    start=(c == 0), stop=(c == RC - 1))
                            if it == 2 and c in (4, 20):  # phase-3 x prefetch
                                hb3 = {4: 4, 20: 5}[c]
                                x3[hb3] = xpool.tile([128, XH, 2, N], f16,
                                                     tag="x", name=f"x3_{hb3}")
                                nc.sync.dma_start(out=x3[hb3], in_=xT_hb[hb3])
                        if it == 2:
                            for h in range(2):
                                for ita in range(2):
                                    softmax(dotsA[h][ita], attn[h][:, ita, :])
                            for h in range(2):
                                nc.sync.dma_start_transpose(
                                    out=attnT[h][:, 0:2], in_=attn[h][:, 0:2, :])
                        for h in range(2):
                            softmax(dotsB[h][it], attn[h][:, it, :])
                        for h in range(2):
                            nc.sync.dma_start_transpose(
                                out=attnT[h][:, it:it + 1],
                                in_=attn[h][:, it:it + 1, :])

        # ------- Phase 3: v, out, outT (PE transpose), y ------------------
        # The out^T transposes run on the PE itself (is_transpose matmuls,
        # 53ns per 128x128 fp16 tile): no DMA-server or cross-queue latency
        # in the B -> y critical chain, so stage lags are short.  PSUM: v(2)
        # + out(2) + T fp16(2 half-banks) + 2 y half-tiles = 8 banks.
        with tc.tile_pool(name="vpool", bufs=12) as vpool, \
             tc.tile_pool(name="outp", bufs=5) as outp, \
             tc.tile_pool(name="outtp", bufs=5) as outtp, \
             tc.tile_pool(name="ypool", bufs=2) as ypool:
            ident = singles.tile([128, 128], f16, name="ident")
            nc.scalar.dma_start(out=ident, in_=idm[:, :])
            v_sbs = {}
            out_sbs = {}
            outT_sbs = {}
            y_sbs = {}

            def stage_a(r):
                hb, ri = divmod(r, XH)
                # issue-ahead of 4 halves: the reused buffer slot's readers
                # retired ~9 rows ago, so this DMA never parks on the SP queue
                if ri == 0 and 6 <= hb + 4 < NXH:
                    x3[hb + 4] = xpool.tile([128, XH, 2, N], f16, tag="x",
                                            name=f"x3_{hb + 4}")
                    nc.sync.dma_start(out=x3[hb + 4], in_=xT_hb[hb + 4])
                v_ps = ps.tile([128, 4, 128], f32, tag="dots", bufs=4,
                                name=f"v_ps_{r}")
                for jt in range(4):
                    for cc in range(2):
                        nc.tensor.matmul(
                            v_ps[:, jt, :],
                            lhsT=x3[hb][:, ri, cc, jt * 128:(jt + 1) * 128],
                            rhs=wv_sb[:, cc, :],
                            start=(cc == 0), stop=(cc == 1))
                v_sb = vpool.tile([128, 4, 128], f16, tag="vsb", name=f"v_sb_{r}")
                # entry rows copy on DVE (idle until B(0)) so the A-lead can
                # run ahead while ACT drains softmax B
                if r < 9:
                    nc.vector.tensor_copy(v_sb, v_ps)
                else:
                    nc.scalar.copy(v_sb, v_ps)
                v_sbs[r] = v_sb

            out_pss = {}

            def stage_b(r, its=(0, 1, 2, 3), done=True):
                if r in out_pss:
                    out_ps = out_pss[r]
                else:
                    # entry rows 0-1 park on the q-tag banks (freed by the
                    # early it2 softmax) so the long-lived entry out tiles
                    # don't stall the dots-tag v/out rotation
                    tg, bf = ("q", 2) if r < 2 and its != (0, 1, 2, 3) \
                        else ("dots", 4)
                    out_ps = out_pss[r] = ps.tile(
                        [128, 4, 128], f32, tag=tg, bufs=bf,
                        name=f"out_ps_{r}")
                for it in its:
                    for h in range(2):
                        hs = slice(h * 64, (h + 1) * 64)
                        for jc in range(4):
                            nc.tensor.matmul(
                                out_ps[:, it, hs],
                                lhsT=attnT[h][:, it, jc, :],
                                rhs=v_sbs[r][:, jc, hs],
                                start=(jc == 0), stop=(jc == 3),
                                skip_group_check=True)
                if done:
                    v_sbs.pop(r)
                    out_pss.pop(r)
                    out_sb = outp.tile([128, 4, 128], f16, tag="ob",
                                       name=f"out_sb_{r}")
                    nc.vector.tensor_copy(out_sb, out_ps)
                    out_sbs[r] = out_sb

            def stage_t(r):
                out_sb = out_sbs.pop(r)
                # padded to a full 2KB PSUM bank so the shared "q" tag
                # keeps a single tile size
                t_ps = ps.tile([128, 8, 128], f16, tag="q", name=f"t_ps_{r}")
                for it in range(4):
                    nc.tensor.transpose(t_ps[:, it, :], out_sb[:, it, :], ident)
                outT = outtp.tile([128, 4, 128], f16, tag="ot",
                                  name=f"outT_{r}")
                if r >= R - 4:  # drain: ACT is the serializer, DVE is free
                    nc.vector.tensor_copy(outT, t_ps[:, 0:4, :])
                else:
                    nc.scalar.copy(outT, t_ps[:, 0:4, :])
                outT_sbs[r] = outT

            def stage_c(r):
                rb, ri = divmod(r, RB)
                outT = outT_sbs.pop(r)
                if ri == 0:
                    y_sbs[rb] = ypool.tile([128, RB, 4, E], f16, tag="ysb",
                                           name=f"y_sb_{rb}")
                y_sb = y_sbs[rb]
                # two 1-bank PSUM halves; copies split ACT/Pool (off DVE so
                # out/T copies flow without queueing delay)
                # drain rows borrow the freed dots/q banks so the final
                # C stages double-buffer instead of chaining on 2 banks
                ta, tb = ("dots", "q") if r >= R - 5 else ("k", "k")
                y_psa = ps.tile([128, 2, E], f32, tag=ta,
                                bufs=4 if r >= R - 5 else 2,
                                name=f"y_psa_{r}")
                for it in range(2):
                    nc.tensor.matmul(y_psa[:, it, :], lhsT=outT[:, it, :],
                                     rhs=wo_sb, start=True, stop=True)
                nc.vector.tensor_copy(y_sb[:, ri, 0:2, :], y_psa)
                y_psb = ps.tile([128, 2, E], f32, tag=tb, name=f"y_psb_{r}")
                for it in range(2, 4):
                    nc.tensor.matmul(y_psb[:, it - 2, :], lhsT=outT[:, it, :],
                                     rhs=wo_sb, start=True, stop=True)
                nc.scalar.copy(y_sb[:, ri, 2:4, :], y_psb)
                # half-block writeout on the SWDGE (Pool) queue; the last
                # block drains in 2-row pieces, the final ones on the idle
                # SP/HWDGE queue (~1.2us less SWDGE generation each)
                if rb == RB - 1:
                    pieces = {i: i for i in range(RB)}  # per-row pieces
                else:
                    pieces = {3: 0, 7: 4}
                if ri in pieces:
                    lo = pieces[ri]
                    if rb == RB - 1:
                        # SP/HWDGE for the whole last block: ~1.5us less
                        # SWDGE generation latency per piece at the end
                        nc.sync.dma_start(out=y_blk[rb, :, lo:ri + 1],
                                          in_=y_sb[:, lo:ri + 1])
                    else:
                        nc.gpsimd.dma_start(out=y_blk[rb, :, lo:ri + 1],
                                            in_=y_sb[:, lo:ri + 1])
                    if ri == RB - 1:
                        y_sbs.pop(rb)

            # stage A leads by 9 rows: the v projections (independent of
            # attnT) keep the PE busy through the softmax-B -> attnT
            # transpose latency at phase-3 entry
            for r in range(R + 12):
                if r < R:
                    stage_a(r)
                # entry: i-tile-0/1 halves of rows 0-3 run first (they need
                # only the early attnT transposes), the 2/3 halves catch up
                # two per iteration once the late transposes land
                if r == 9:
                    stage_b(0, its=(0, 1), done=False)
                    stage_b(1, its=(0, 1), done=False)
                elif r == 10:
                    stage_b(2, its=(0, 1), done=False)
                    stage_b(3, its=(0, 1), done=False)
                elif r == 11:
                    # all four it2 halves before any it3 half: the in-order
                    # PE stream then stalls only once, on the it3 attnT
                    for q_ in range(4):
                        stage_b(q_, its=(2,), done=False)
                elif r == 12:
                    for q_ in range(4):
                        stage_b(q_, its=(3,))
                elif 0 <= r - 9 < R:
                    stage_b(r - 9)
                if r == 13:
                    stage_t(0)
                    stage_t(1)
                elif r == 14:
                    stage_t(2)
                    stage_t(3)
                elif 0 <= r - 11 < R - 4 and r - 11 >= 4:
                    stage_t(r - 11)
                if R - 4 <= r - 10 < R:
                    stage_t(r - 10)  # tail rows: 1-iter shorter lag
                if r == 14:
                    stage_c(0)
                    stage_c(1)
                elif r == 15:
                    stage_c(2)
                    stage_c(3)
                elif 0 <= r - 12 < R - 4 and r - 12 >= 4:
                    stage_c(r - 12)
                if R - 4 <= r - 11 < R:
                    stage_c(r - 11)

    return nc


def _get_program():
    if "nc" not in _CACHE:
        nc = build_program()
        nc.finalize()
        _CACHE["nc"] = nc
    return _CACHE["nc"]


def make_in_maps(x, Wq, Wkv, Wo):
    """Host-side sharding: core = bi*4 + hpi."""
    scale = (64.0 ** -0.5) * (64.0 ** -0.5)
    x = np.asarray(x, np.float32)
    Wq = np.asarray(Wq, np.float32) * np.float32(scale)
    Wkv = np.asarray(Wkv, np.float32)
    Wo = np.asarray(Wo, np.float32)
    b = x.shape[0] // R
    xT = np.ascontiguousarray(
        x.reshape(b, R, N, C).transpose(0, 1, 3, 2)).astype(np.float16)

    def swap_heads(w):  # [C, 128] -> column halves swapped
        return np.ascontiguousarray(
            np.concatenate([w[:, 64:], w[:, :64]], axis=1))

    in_maps = []
    for core in range(NCORES):
        bi, hpi = divmod(core, 4)
        cols = slice(hpi * HP, (hpi + 1) * HP)
        wq_c = np.ascontiguousarray(Wq[:, cols]).astype(np.float16)
        wk_c = np.ascontiguousarray(Wkv[:, cols]).astype(np.float16)
        in_maps.append({
            "xT": xT[bi],
            "idm": np.eye(128, dtype=np.float16),
            "wqe": wq_c,
            "wqo": swap_heads(wq_c),
            "wke": wk_c,
            "wko": swap_heads(wk_c),
            "wv": np.ascontiguousarray(
                Wkv[:, 512 + hpi * HP: 512 + (hpi + 1) * HP]).astype(np.float16),
            "wo": np.ascontiguousarray(Wo[cols, :]).astype(np.float16),
        })
    return in_maps


def combine_outputs(ys, bo):
    """ys: list of 8 [R, N, E] partials in core order; returns [B, n, dim]."""
    ys = [np.asarray(t, np.float32) for t in ys]
    y0 = ys[0] + ys[1] + ys[2] + ys[3]
    y1 = ys[4] + ys[5] + ys[6] + ys[7]
    y = np.concatenate([y0, y1], axis=0).reshape(2 * R, N, E)
    return (y + np.asarray(bo, np.float32)).astype(np.float32)


def kernel(x, Wq, Wkv, Wo, bo, tie_attn_dim):
    assert int(tie_attn_dim) == R, f"hardcoded for tie_attn_dim={R}"
    from concourse.bass_utils import run_bass_kernel_spmd

    nc = _get_program()
    in_maps = make_in_maps(x, Wq, Wkv, Wo)
    res = run_bass_kernel_spmd(nc, in_maps, list(range(NCORES)))
    ys = [res.results[c]["y"] for c in range(NCORES)]
    return combine_outputs(ys, bo)


# revision 79
# speedup vs baseline: 1.0034x; 1.0026x over previous
"""Tied-row (MSA) attention on 8 Trainium2 NeuronCores.

Reference computation (B=128, n=512, dim=256, h=8, dh=64,
r=tie_attn_dim=64, b=B//r=2):
    q = x @ Wq ; k,v = split(x @ Wkv)
    dots[b,h,i,j] = sum_{r,d} q[b,r,h,i,d] k[b,r,h,j,d] * scale
    attn = softmax_j(dots)
    out[b,r,h,i,d] = sum_j attn[b,h,i,j] v[b,r,h,j,d]
    y = out @ Wo + bo

Sharding: 8 cores = b(2) x head-pairs(4).  Each core owns one batch
element and 2 of the 8 heads and produces the partial
    y_part = out[:, :, own 2 heads, :] @ Wo[own 128 rows, :]
summed on the host (the head reduction of the output projection
commutes with the sum); bo added once.

Cost-model-driven design (timeline cost = sum over matmuls of OUTPUT
FREE SIZE x 0.4167ns; K and M are free):
  * dots contracts K=128 = (2 MSA rows x 64 d) per accumulation step
    (32 steps instead of 64), halving the dots charge.  q/k are staged
    per-head as [128=(parity,d), 32 rchunk, n] fp16.  Engine copies
    cannot cross partitions, so the projection alternates head-swapped
    weight copies (wq_even / wq_odd with column halves swapped) making
    every split copy partition-identity:
      even r: ps[0:64]=h0 -> q0[0:64], ps[64:128]=h1 -> q1[64:128]
      odd  r: ps[0:64]=h1 -> q1[0:64], ps[64:128]=h0 -> q0[64:128]
    q goes PSUM -> f16 staging (DVE) then two cheap f16 splits (DVE 2x
    + Pool); k copies direct on ACT (Pool cannot read PSUM on walrus).
  * out is computed as [i, (h d)] (lhsT=attnT chunk, rhs=v[j,(h,d)]
    slice) so the charged free axis is d=64: 32 matmuls x 64 free per
    row instead of 8 x 512.  The y projection needs out^T [(h d), i];
    those transposes run on the PE itself (nc.tensor.transpose with a
    DRAM-loaded identity, 53ns per 128x128 fp16 tile) -- no DMA-server
    or cross-queue latency in the B -> y critical chain.
  * y is written fp16 (halves the writeout on the single exclusive
    DMA server, elem=512B full rate); partials summed in f32 on host.
  * one kernel-wide PSUM pool with shared tags (pool-scope closes
    would barrier phase 3 on all softmax reads): dots tag = dotsA +
    phase-3 v/out rotation, q tag = q_ps + t_ps, k tag = k_ps + dotsB
    + y halves.
  * x streams as 4-row half-block DMAs; phase-3 halves 0-5 prefetch
    into phase-1/2 DMA slack, the rest issue 4 ahead (never parking on
    the in-order SP queue); y writeout on the Pool/SWDGE queue keeps
    SP free for x (no head-of-line inversion into PE-critical loads).

Per-core phases (PE charge 232.2us = 218.5 floor + 13.7 PE transpose):
  Phase 1 (r-loop, proj + fused dots wave A for i-tiles 0,1): softmax
    A and its attnT transposes overlap wave B.
  Wave B (i-tiles 2,3) reuses the q/k PSUM banks; softmax B + per-head
    late attnT transposes feed phase 3.
  Phase 3 (stages A=v proj, B=out, T=PE transpose, C=y, lags 9/11/12;
    stage A leads 9 rows so v projections cover the attnT latency at
    entry; rows 0-3 run their i-tile-0/1 halves first).  Last block
    drains in 2-row transposes and 2/1-row y DMAs to shorten the tail.

  Built with bacc.Bacc(): its compile() pass legalizes Tile's sync for
  walrus (which caps sync waits per instruction); callers must
  finalize() the program before running (see _get_program).
"""

import os
import sys

for _p in ("/opt/trn_rl_repo", "/root/.axon_site/_ro/trn_rl_repo"):
    if os.path.isdir(_p) and _p not in sys.path:
        sys.path.insert(0, _p)

import numpy as np

R = 64          # tie dim (MSA rows per batch element)
RB = 8          # rows per DMA block
RC = R // 2     # dots K-chunks (2 rows each)
N = 512         # sequence length
C = 256         # model dim
HP = 128        # head-pair width: 2 heads x 64
E = 256         # output dim
NCORES = 8
CLAG = 8        # phase-3 stage-C lag behind stage B (rows)
XH = 4          # rows per x half-block tile
NXH = R // XH   # 16 x half-blocks

_CACHE = {}


def build_program():
    import concourse.bacc as bacc
    from concourse import mybir
    from concourse.tile import TileContext
    from contextlib import ExitStack

    f32 = mybir.dt.float32
    f16 = mybir.dt.float16

    nc = bacc.Bacc()
    xT = nc.declare_dram_parameter("xT", [R, C, N], f16, isOutput=False)
    wqe = nc.declare_dram_parameter("wqe", [C, HP], f16, isOutput=False)
    wqo = nc.declare_dram_parameter("wqo", [C, HP], f16, isOutput=False)
    wke = nc.declare_dram_parameter("wke", [C, HP], f16, isOutput=False)
    wko = nc.declare_dram_parameter("wko", [C, HP], f16, isOutput=False)
    wv = nc.declare_dram_parameter("wv", [C, HP], f16, isOutput=False)
    wo = nc.declare_dram_parameter("wo", [HP, E], f16, isOutput=False)
    idm = nc.declare_dram_parameter("idm", [128, 128], f16, isOutput=False)
    y = nc.declare_dram_parameter("y", [R, N, E], f16, isOutput=True)

    # xT half-block hb viewed as [p, r_in_half, c_chunk, n]
    xT_hb = xT.rearrange("(hb r) (cc p) n -> hb p r cc n", r=XH, p=128)
    # y block rb viewed as [p, r_in_block, i_tile, e]
    y_blk = y.rearrange("(rb r) (t p) e -> rb p r t e", r=RB, p=128)

    with TileContext(nc) as tc, ExitStack() as ctx:
        singles = ctx.enter_context(tc.tile_pool(name="singles", bufs=1))
        sm = ctx.enter_context(tc.tile_pool(name="sm", bufs=4))
        attntp = ctx.enter_context(tc.tile_pool(name="attntp", bufs=2))
        xpool = ctx.enter_context(tc.tile_pool(name="xpool", bufs=6))
        stp = ctx.enter_context(tc.tile_pool(name="stp", bufs=2))
        # one PSUM pool for the whole kernel: pool-scope closes would force
        # phase-3 bank allocations to wait on ALL phase-1/2 readers; with
        # shared tags phase 3 lands on the banks softmax A freed early
        ps = ctx.enter_context(tc.tile_pool(name="ps", space="PSUM", bufs=2))

        # weights first, one DMA each, split across the two HWDGE queues
        # (SP + ACT) so they land in ~2us; then x block 0 in 2-row slices
        # so the first projection starts ~2us after that.
        wq_sb = [singles.tile([128, 2, HP], f16, name=f"wq{p}") for p in range(2)]
        wk_sb = [singles.tile([128, 2, HP], f16, name=f"wk{p}") for p in range(2)]
        wv_sb = singles.tile([128, 2, HP], f16)
        wo_sb = singles.tile([128, E], f16)
        # x rows 0-3 race in first on both queues, then weights, x half 1,
        # and the phase-3-only wv/wo
        w2 = "(cc p) h -> p cc h"
        x_sb0 = xpool.tile([128, XH, 2, N], f16, tag="x", name="x1_0")
        nc.sync.dma_start(out=x_sb0[:, 0:2], in_=xT_hb[0, :, 0:2])
        nc.scalar.dma_start(out=wq_sb[1], in_=wqo.rearrange(w2, p=128))
        nc.sync.dma_start(out=wq_sb[0], in_=wqe.rearrange(w2, p=128))
        nc.scalar.dma_start(out=wk_sb[1], in_=wko.rearrange(w2, p=128))
        nc.sync.dma_start(out=wk_sb[0], in_=wke.rearrange(w2, p=128))
        nc.scalar.dma_start(out=x_sb0[:, 2:4], in_=xT_hb[0, :, 2:4])
        x_sb1 = xpool.tile([128, XH, 2, N], f16, tag="x", name="x1_1")
        nc.sync.dma_start(out=x_sb1[:, 0:2], in_=xT_hb[1, :, 0:2])
        nc.scalar.dma_start(out=x_sb1[:, 2:4], in_=xT_hb[1, :, 2:4])
        nc.scalar.dma_start(out=wv_sb, in_=wv.rearrange(w2, p=128))
        nc.scalar.dma_start(out=wo_sb, in_=wo[:, :])

        # attnT survives into phase 3: kernel-scoped pool
        # layout [j_in_chunk, it, jc, i_in_tile]
        attnT = [attntp.tile([128, 4, 4, 128], f16, tag="attnT",
                             name=f"attnT_{h}") for h in range(2)]

        def softmax(dots_hit, attn_dst):
            """dots PSUM tile -> normalized f16 attn slice.

            No max-subtraction: dots = q k^T with the 1/(sqrt(dh) sqrt(r))
            scale folded into Wq, so entries are ~N(0,1) and exp cannot
            overflow fp32/fp16."""
            ssum = sm.tile([128, 1], f32, tag="ssum", bufs=8)
            rinv = sm.tile([128, 1], f32, tag="rinv", bufs=8)
            nc.scalar.activation(
                out=attn_dst, in_=dots_hit,
                func=mybir.ActivationFunctionType.Exp,
                accum_out=ssum)
            nc.vector.reciprocal(rinv, ssum)
            nc.vector.tensor_scalar_mul(attn_dst, attn_dst, rinv)

        if True:
            # attn_h [i_in_tile, it, j]; one tile per head so the transpose
            # to attnT_h is a single xbar DMA per head (kernel-scoped pool:
            # a pool close before phase 3 would barrier DVE on the late
            # attn transposes)
            attn = [attntp.tile([128, 4, N], f16, name=f"attn_{h}")
                    for h in range(2)]

            # resident per-head K-packed fp16 q/k, phases 1-2 only
            with tc.tile_pool(name="resid", bufs=1) as resid:
                qh = [resid.tile([128, RC, N], f16, name=f"q{h}")
                      for h in range(2)]
                kh = [resid.tile([128, RC, N], f16, name=f"k{h}")
                      for h in range(2)]

                def dots_chunk(dots_tiles, c, its):
                    for h in range(2):
                        for it in its:
                            nc.tensor.matmul(
                                dots_tiles[h][it % 2],
                                lhsT=qh[h][:, c, it * 128:(it + 1) * 128],
                                rhs=kh[h][:, c, :],
                                start=(c == 0), stop=(c == RC - 1))

                # ---- Phase 1 + dots wave A (i-tiles 0,1) fused;
                # ---- wave B (i-tiles 2,3) reuses the q/k PSUM banks and
                # ---- runs before softmax A is emitted so the ACT exps of
                # ---- wave A overlap wave B's accumulation.
                x3 = {}
                if True:
                    dotsA = [[ps.tile([128, N], f32, tag="dots", bufs=4,
                                       name=f"dotsA_{h}_{it}")
                              for it in range(2)] for h in range(2)]
                    x_sb = x_sb0
                    for r in range(R):
                        hb, ri = divmod(r, XH)
                        if ri == 0 and hb == 1:
                            x_sb = x_sb1
                        elif ri == 0 and hb > 1:
                            x_sb = xpool.tile([128, XH, 2, N], f16, tag="x",
                                              name=f"x1_{hb}")
                            nc.sync.dma_start(out=x_sb, in_=xT_hb[hb])
                        # prefetch phase-3 x halves 0-3 into the slots the
                        # phase-1 stream has finished with
                        if r >= 48 and ri == 0:
                            hb3 = (r - 48) // 4
                            x3[hb3] = xpool.tile([128, XH, 2, N], f16, tag="x",
                                                 name=f"x3_{hb3}")
                            nc.sync.dma_start(out=x3[hb3], in_=xT_hb[hb3])
                        par = r % 2
                        rc = r // 2
                        q_ps = ps.tile([128, N], f32, tag="q")
                        k_ps = ps.tile([128, N], f32, tag="k")
                        for cc in range(2):
                            nc.tensor.matmul(q_ps, lhsT=wq_sb[par][:, cc, :],
                                             rhs=x_sb[:, ri, cc, :],
                                             start=(cc == 0), stop=(cc == 1))
                        for cc in range(2):
                            nc.tensor.matmul(k_ps, lhsT=wk_sb[par][:, cc, :],
                                             rhs=x_sb[:, ri, cc, :],
                                             start=(cc == 0), stop=(cc == 1))
                        # partition-identity split copies (see module doc):
                        # lo half -> head `par`, hi half -> head `1-par`.
                        # q goes PSUM -> f16 staging (DVE 1x) then two cheap
                        # f16 SBUF splits (DVE 2x mode + Pool), keeping every
                        # engine under the phase-1 PE time; k copies direct
                        # on ACT.  (Pool cannot read PSUM on walrus.)
                        lo, hi = (0, 1) if par == 0 else (1, 0)
                        q_st = stp.tile([128, N], f16, tag="qst", name=f"qst_{r}")
                        nc.vector.tensor_copy(q_st, q_ps)
                        nc.vector.tensor_copy(
                            qh[lo][0:64, rc, :], q_st[0:64, :])
                        nc.gpsimd.tensor_copy(
                            qh[hi][64:128, rc, :], q_st[64:128, :])
                        nc.scalar.copy(kh[lo][0:64, rc, :], k_ps[0:64, :])
                        nc.scalar.copy(kh[hi][64:128, rc, :], k_ps[64:128, :])
                        # wave A dots, two rows behind the copies
                        if par == 1 and r >= 3:
                            dots_chunk(dotsA, (r - 3) // 2, (0, 1))
                    dots_chunk(dotsA, RC - 1, (0, 1))

                    # wave B accumulators take over the q/k bank slots
                    dotsB = [[ps.tile([128, N], f32, tag=t, name=f"dotsB_{h}_{it}")
                              for it, t in ((2, "q"), (3, "k"))] for h in range(2)]
                    dotsB = [{2: dotsB[h][0], 3: dotsB[h][1]} for h in range(2)]
                    # wave B runs as two sequential sub-waves so i-tile
                    # 2's softmax + transpose complete ~13us before the wave
                    # ends -- only i-tile 3's short chain stays on the
                    # phase-3 critical path.  Softmax A and its transposes
                    # overlap sub-wave it2 on ACT/DVE/SP.
                    # each (it, h) tile accumulates sequentially and its
                    # softmax + transpose emit immediately: h0-it3's chain
                    # completes mid-wave, so only h1-it3 trails the wave end
                    for it in (2, 3):
                        for h in range(2):
                            for c in range(RC):
                                nc.tensor.matmul(
                                    dotsB[h][it],
                                    lhsT=qh[h][:, c, it * 128:(it + 1) * 128],
                                    rhs=kh[h][:, c, :],
                                    start=(c == 0), stop=(c == RC - 1))
                                if it == 2 and h == 0 and c in (4, 20):
                                    hb3 = {4: 4, 20: 5}[c]  # phase-3 x prefetch
                                    x3[hb3] = xpool.tile(
                                        [128, XH, 2, N], f16,
                                        tag="x", name=f"x3_{hb3}")
                                    nc.sync.dma_start(out=x3[hb3],
                                                      in_=xT_hb[hb3])
                            softmax(dotsB[h][it], attn[h][:, it, :])
                            nc.sync.dma_start_transpose(
                                out=attnT[h][:, it:it + 1],
                                in_=attn[h][:, it:it + 1, :])
                        if it == 2:
                            for h in range(2):
                                for ita in range(2):
                                    softmax(dotsA[h][ita], attn[h][:, ita, :])
                            for h in range(2):
                                nc.sync.dma_start_transpose(
                                    out=attnT[h][:, 0:2], in_=attn[h][:, 0:2, :])

        # ------- Phase 3: v, out, outT (PE transpose), y ------------------
        # The out^T transposes run on the PE itself (is_transpose matmuls,
        # 53ns per 128x128 fp16 tile): no DMA-server or cross-queue latency
        # in the B -> y critical chain, so stage lags are short.  PSUM: v(2)
        # + out(2) + T fp16(2 half-banks) + 2 y half-tiles = 8 banks.
        with tc.tile_pool(name="vpool", bufs=12) as vpool, \
             tc.tile_pool(name="outp", bufs=5) as outp, \
             tc.tile_pool(name="outtp", bufs=5) as outtp, \
             tc.tile_pool(name="ypool", bufs=2) as ypool:
            ident = singles.tile([128, 128], f16, name="ident")
            nc.scalar.dma_start(out=ident, in_=idm[:, :])
            v_sbs = {}
            out_sbs = {}
            outT_sbs = {}
            y_sbs = {}

            def stage_a(r):
                hb, ri = divmod(r, XH)
                # issue-ahead of 4 halves: the reused buffer slot's readers
                # retired ~9 rows ago, so this DMA never parks on the SP queue
                if ri == 0 and 6 <= hb + 4 < NXH:
                    x3[hb + 4] = xpool.tile([128, XH, 2, N], f16, tag="x",
                                            name=f"x3_{hb + 4}")
                    nc.sync.dma_start(out=x3[hb + 4], in_=xT_hb[hb + 4])
                v_ps = ps.tile([128, 4, 128], f32, tag="dots", bufs=4,
                                name=f"v_ps_{r}")
                for jt in range(4):
                    for cc in range(2):
                        nc.tensor.matmul(
                            v_ps[:, jt, :],
                            lhsT=x3[hb][:, ri, cc, jt * 128:(jt + 1) * 128],
                            rhs=wv_sb[:, cc, :],
                            start=(cc == 0), stop=(cc == 1))
                v_sb = vpool.tile([128, 4, 128], f16, tag="vsb", name=f"v_sb_{r}")
                # entry rows copy on DVE (idle until B(0)) so the A-lead can
                # run ahead while ACT drains softmax B
                if r < 9:
                    nc.vector.tensor_copy(v_sb, v_ps)
                else:
                    nc.scalar.copy(v_sb, v_ps)
                v_sbs[r] = v_sb

            out_pss = {}

            def stage_b(r, its=(0, 1, 2, 3), done=True):
                if r in out_pss:
                    out_ps = out_pss[r]
                else:
                    # entry rows 0-1 park on the q-tag banks (freed by the
                    # early it2 softmax) so the long-lived entry out tiles
                    # don't stall the dots-tag v/out rotation
                    tg, bf = ("q", 2) if r < 2 and its != (0, 1, 2, 3) \
                        else ("dots", 4)
                    out_ps = out_pss[r] = ps.tile(
                        [128, 4, 128], f32, tag=tg, bufs=bf,
                        name=f"out_ps_{r}")
                for it in its:
                    for h in range(2):
                        hs = slice(h * 64, (h + 1) * 64)
                        for jc in range(4):
                            nc.tensor.matmul(
                                out_ps[:, it, hs],
                                lhsT=attnT[h][:, it, jc, :],
                                rhs=v_sbs[r][:, jc, hs],
                                start=(jc == 0), stop=(jc == 3),
                                skip_group_check=True)
                if done:
                    v_sbs.pop(r)
                    out_pss.pop(r)
                    out_sb = outp.tile([128, 4, 128], f16, tag="ob",
                                       name=f"out_sb_{r}")
                    nc.vector.tensor_copy(out_sb, out_ps)
                    out_sbs[r] = out_sb

            def stage_t(r):
                out_sb = out_sbs.pop(r)
                # padded to a full 2KB PSUM bank so the shared "q" tag
                # keeps a single tile size
                t_ps = ps.tile([128, 8, 128], f16, tag="q", name=f"t_ps_{r}")
                for it in range(4):
                    nc.tensor.transpose(t_ps[:, it, :], out_sb[:, it, :], ident)
                outT = outtp.tile([128, 4, 128], f16, tag="ot",
                                  name=f"outT_{r}")
                if r >= R - 4:  # drain: ACT is the serializer, DVE is free
                    nc.vector.tensor_copy(outT, t_ps[:, 0:4, :])
                else:
                    nc.scalar.copy(outT, t_ps[:, 0:4, :])
                outT_sbs[r] = outT

            def stage_c(r):
                rb, ri = divmod(r, RB)
                outT = outT_sbs.pop(r)
                if ri == 0:
                    y_sbs[rb] = ypool.tile([128, RB, 4, E], f16, tag="ysb",
                                           name=f"y_sb_{rb}")
                y_sb = y_sbs[rb]
                # two 1-bank PSUM halves; copies split ACT/Pool (off DVE so
                # out/T copies flow without queueing delay)
                # drain rows borrow the freed dots/q banks so the final
                # C stages double-buffer instead of chaining on 2 banks
                ta, tb = ("dots", "q") if r >= R - 5 else ("k", "k")
                y_psa = ps.tile([128, 2, E], f32, tag=ta,
                                bufs=4 if r >= R - 5 else 2,
                                name=f"y_psa_{r}")
                for it in range(2):
                    nc.tensor.matmul(y_psa[:, it, :], lhsT=outT[:, it, :],
                                     rhs=wo_sb, start=True, stop=True)
                nc.vector.tensor_copy(y_sb[:, ri, 0:2, :], y_psa)
                y_psb = ps.tile([128, 2, E], f32, tag=tb, name=f"y_psb_{r}")
                for it in range(2, 4):
                    nc.tensor.matmul(y_psb[:, it - 2, :], lhsT=outT[:, it, :],
                                     rhs=wo_sb, start=True, stop=True)
                nc.scalar.copy(y_sb[:, ri, 2:4, :], y_psb)
                # half-block writeout on the SWDGE (Pool) queue; the last
                # block drains in 2-row pieces, the final ones on the idle
                # SP/HWDGE queue (~1.2us less SWDGE generation each)
                if rb == RB - 1:
                    pieces = {i: i for i in range(RB)}  # per-row pieces
                else:
                    pieces = {3: 0, 7: 4}
                if ri in pieces:
                    lo = pieces[ri]
                    if rb == RB - 1:
                        # SP/HWDGE for the whole last block: ~1.5us less
                        # SWDGE generation latency per piece at the end
                        nc.sync.dma_start(out=y_blk[rb, :, lo:ri + 1],
                                          in_=y_sb[:, lo:ri + 1])
                    else:
                        nc.gpsimd.dma_start(out=y_blk[rb, :, lo:ri + 1],
                                            in_=y_sb[:, lo:ri + 1])
                    if ri == RB - 1:
                        y_sbs.pop(rb)

            # stage A leads by 9 rows: the v projections (independent of
            # attnT) keep the PE busy through the softmax-B -> attnT
            # transpose latency at phase-3 entry
            for r in range(R + 12):
                if r < R:
                    stage_a(r)
                # entry: i-tile-0/1 halves of rows 0-3 run first (they need
                # only the early attnT transposes), the 2/3 halves catch up
                # two per iteration once the late transposes land
                if r == 9:
                    stage_b(0, its=(0, 1), done=False)
                    stage_b(1, its=(0, 1), done=False)
                elif r == 10:
                    stage_b(2, its=(0, 1), done=False)
                    stage_b(3, its=(0, 1), done=False)
                elif r == 11:
                    # all four it2 halves before any it3 half: the in-order
                    # PE stream then stalls only once, on the it3 attnT
                    for q_ in range(4):
                        stage_b(q_, its=(2,), done=False)
                elif r == 12:
                    for q_ in range(4):
                        stage_b(q_, its=(3,))
                elif 0 <= r - 9 < R:
                    stage_b(r - 9)
                if r == 13:
                    stage_t(0)
                    stage_t(1)
                elif r == 14:
                    stage_t(2)
                    stage_t(3)
                elif 0 <= r - 11 < R - 4 and r - 11 >= 4:
                    stage_t(r - 11)
                if R - 4 <= r - 10 < R:
                    stage_t(r - 10)  # tail rows: 1-iter shorter lag
                if r == 14:
                    stage_c(0)
                    stage_c(1)
                elif r == 15:
                    stage_c(2)
                    stage_c(3)
                elif 0 <= r - 12 < R - 4 and r - 12 >= 4:
                    stage_c(r - 12)
                if R - 4 <= r - 11 < R:
                    stage_c(r - 11)

    return nc


def _get_program():
    if "nc" not in _CACHE:
        nc = build_program()
        nc.finalize()
        _CACHE["nc"] = nc
    return _CACHE["nc"]


def make_in_maps(x, Wq, Wkv, Wo):
    """Host-side sharding: core = bi*4 + hpi."""
    scale = (64.0 ** -0.5) * (64.0 ** -0.5)
    x = np.asarray(x, np.float32)
    Wq = np.asarray(Wq, np.float32) * np.float32(scale)
    Wkv = np.asarray(Wkv, np.float32)
    Wo = np.asarray(Wo, np.float32)
    b = x.shape[0] // R
    xT = np.ascontiguousarray(
        x.reshape(b, R, N, C).transpose(0, 1, 3, 2)).astype(np.float16)

    def swap_heads(w):  # [C, 128] -> column halves swapped
        return np.ascontiguousarray(
            np.concatenate([w[:, 64:], w[:, :64]], axis=1))

    in_maps = []
    for core in range(NCORES):
        bi, hpi = divmod(core, 4)
        cols = slice(hpi * HP, (hpi + 1) * HP)
        wq_c = np.ascontiguousarray(Wq[:, cols]).astype(np.float16)
        wk_c = np.ascontiguousarray(Wkv[:, cols]).astype(np.float16)
        in_maps.append({
            "xT": xT[bi],
            "idm": np.eye(128, dtype=np.float16),
            "wqe": wq_c,
            "wqo": swap_heads(wq_c),
            "wke": wk_c,
            "wko": swap_heads(wk_c),
            "wv": np.ascontiguousarray(
                Wkv[:, 512 + hpi * HP: 512 + (hpi + 1) * HP]).astype(np.float16),
            "wo": np.ascontiguousarray(Wo[cols, :]).astype(np.float16),
        })
    return in_maps


def combine_outputs(ys, bo):
    """ys: list of 8 [R, N, E] partials in core order; returns [B, n, dim]."""
    ys = [np.asarray(t, np.float32) for t in ys]
    y0 = ys[0] + ys[1] + ys[2] + ys[3]
    y1 = ys[4] + ys[5] + ys[6] + ys[7]
    y = np.concatenate([y0, y1], axis=0).reshape(2 * R, N, E)
    return (y + np.asarray(bo, np.float32)).astype(np.float32)


def kernel(x, Wq, Wkv, Wo, bo, tie_attn_dim):
    assert int(tie_attn_dim) == R, f"hardcoded for tie_attn_dim={R}"
    from concourse.bass_utils import run_bass_kernel_spmd

    nc = _get_program()
    in_maps = make_in_maps(x, Wq, Wkv, Wo)
    res = run_bass_kernel_spmd(nc, in_maps, list(range(NCORES)))
    ys = [res.results[c]["y"] for c in range(NCORES)]
    return combine_outputs(ys, bo)
